# revision 18
# baseline (speedup 1.0000x reference)
"""BioZorro sparse-attention kernel for 8 Trainium2 NeuronCores.

Sharding: 8 cores = 2 batches x 4 token-quarters (384 own tokens each).
The zorro mask makes all non-fusion query rows fully masked -> uniform
softmax -> their attention output is mean(V) over all tokens; only the 16
fusion tokens do real attention (over the 1536 non-fusion keys). Fusion
rows are replicated on all 4 cores of a batch; cross-core data is a tiny
per-layer AllGather of V column sums (early) plus one of flash-softmax
partials, and one small AllGather for the final pooling.

Perf structure vs the bf16 baseline:
- Heavy matmuls run fp8e4m3 with DoubleRow perf mode (two 128-row
  contraction tiles per instruction). Scales are powers of two folded
  into host weight prep (wq/wkv/ew/pwkv x256, w1 gate x128 / x1 x8,
  w2 x32), undone for free via exp/gelu activation scales and one
  scalar_tensor_tensor residual add (psO/256 + tok).
- LNs feeding attention projections are DEFERRED: projections consume
  the raw fp8 residual stream immediately; the -mu*rstd rank-1 mean
  correction lands in PSUM via one extra matmul per tile (lhsT = host
  column sums of the quantized weights), and rstd is applied during the
  PSUM->SBUF copy. The stats chain runs concurrently on ACT/DVE/Pool.
- wo folded: wo_h x(1/256) fusion path, wo_n x(1/(N*256)) mean-V path.
- PSUM rings: kv(4x2KB: kt/V/psO share), st(2x1.6K stats), g(2x1.6K),
  qf(4x64B), acc(2x256B), dfT(1x256B) -- 15.5KB of the 16KB budget.
"""
import sys
sys.path.insert(0, "/opt/trn_rl_repo")
import numpy as np
import ml_dtypes

BF = ml_dtypes.bfloat16
F8 = ml_dtypes.float8_e4m3
OWN, FUS, TOK = 384, 16, 400
D, RIN, H, DH, IFF, DEPTH = 512, 1024, 8, 64, 1365, 4
NALL = 1552
B, NR, NA = 2, 768, 768
N_CORES = 8

_built = {}


def build(num_devices=8, use_cc=True):
    key = (num_devices, use_cc)
    if key in _built:
        return _built[key]
    import concourse.tile as tile
    from concourse import bacc, mybir
    from concourse.masks import make_identity

    # Keep Exp resolvable from natural_log_exp_and_others so the Ln/Exp
    # pairs in the LN rstd chain share one ACT table set.
    if not getattr(bacc, "_act_tables_patched", False):
        _orig_gat = bacc.get_activation_tables

        def _patched_gat(arch):
            tabs = _orig_gat(arch)
            exp_t = mybir.ActivationFunctionType.Exp
            for nm, fns in tabs.items():
                if nm != "natural_log_exp_and_others":
                    fns.discard(exp_t)
            return tabs

        bacc.get_activation_tables = _patched_gat
        bacc._act_tables_patched = True

    f32 = mybir.dt.float32
    bf16 = mybir.dt.bfloat16
    f8 = mybir.dt.float8e4
    AF = mybir.ActivationFunctionType
    OP = mybir.AluOpType
    DR = mybir.MatmulPerfMode.DoubleRow

    nc = bacc.Bacc("TRN2", target_bir_lowering=False, debug=False,
                   enable_asserts=True, num_devices=num_devices)

    def din(name, shape, dt=f32):
        return nc.dram_tensor(name, shape, dt, kind="ExternalInput").ap()

    x_t = din("x_t", [RIN, OWN], bf16)
    ew_t = din("emb_w", [RIN, D], f8); ebias_t = din("emb_b", [D, 1])
    eg2 = din("eln2_g", [D, 1]); eb2 = din("eln2_b", [D, 1])
    fus_t = din("fus_t", [D, FUS], bf16)

    wq_t = din("wq", [DEPTH, D, D], f8)
    wkv_t = din("wkv", [DEPTH, D, 2 * D], f8)
    woh_t = din("wo_h", [DEPTH, H, DH, D], bf16)
    won_t = din("wo_n", [DEPTH, D, D], bf16)
    w1_t = din("w1", [DEPTH, D, 2 * 1408], f8)
    w2_t = din("w2", [DEPTH, 1408, D], f8)
    wbar_t = din("wbar", [DEPTH, 3, D], bf16)  # rows: kv_k, kv_v, q colsums
    pq2_t = din("pool_q2", [D, 1])
    pwkv_t = din("pool_wkv", [D, 2 * D], f8)
    pbar_t = din("pool_wbar", [2, D], bf16)  # rows: k-half, v-half colsums
    pwoh_t = din("pool_wo_h", [H, DH, D], bf16)
    pwon_t = din("pool_wo_n", [D, D], bf16)
    out_u = nc.dram_tensor("out_u", [D, 1], f32, kind="ExternalOutput").ap()
    out_f = nc.dram_tensor("out_f", [128, 4], f32, kind="ExternalOutput").ap()

    W2T = 11  # k-tiles of padded IFF (11x128)
    IFFP = 11 * 128  # 1408, zero-padded from 1365

    with tile.TileContext(nc) as tc:
        with tc.tile_pool(name="cst", bufs=1) as cst, \
             tc.tile_pool(name="wp", bufs=2) as wp, \
             tc.tile_pool(name="ac", bufs=2) as ac, \
             tc.tile_pool(name="ps", bufs=2, space="PSUM") as pp, \
             tc.tile_pool(name="dramp", bufs=2, space="DRAM") as dramp:

            ident = cst.tile([128, 128], bf16, name="ident")
            make_identity(nc, ident[:])
            ones128 = cst.tile([128, 1], bf16, name="ones128")
            nc.vector.memset(ones128[:], 1.0)
            ones1 = cst.tile([1, 128], bf16, name="ones1")
            nc.vector.memset(ones1[:], 1.0)
            epsc = cst.tile([128, 1], f32, name="epsc")
            nc.vector.memset(epsc[:], 1e-5)
            oi512 = cst.tile([128, 1], bf16, name="oi512")
            nc.vector.memset(oi512[:], 1.0 / 512)
            oi1024 = cst.tile([128, 1], bf16, name="oi1024")
            nc.vector.memset(oi1024[:], 1.0 / 1024)

            # ---------- helpers ----------
            def ln_stats(xs, T, C, tag, want_mB=False, want_col=False,
                         col_chunks=None):
                """Deferred-LN stats over C*128 features, T tokens.

                xs: C bf16 [128,T] APs (raw pre-LN values; any common
                power-of-2 scale -- rstd absorbs it).
                Returns dict: rB [128,T] bf16 rstd broadcast; mursn [1,T]
                bf16 = -mu*rstd (correction-matmul operand); mBn [128,T]
                bf16 broadcast of mursn (want_mB); rC [128,k] bf16 rstd
                column chunks (want_col).
                """
                oi = {4: oi512, 8: oi1024}[C]
                S = pp.tile([1, T], f32, tag="st", bufs=2, name=f"S{tag}")
                Q = pp.tile([1, T], f32, tag="st", bufs=2, name=f"Q{tag}")
                for c in range(C):
                    nc.tensor.matmul(S[:], oi[:], xs[c],
                                     start=(c == 0), stop=(c == C - 1))
                for c in range(C):
                    x2 = ac.tile([128, T], bf16, tag="lnx2", bufs=3,
                                 name="lnx2")
                    nc.vector.tensor_mul(out=x2[:], in0=xs[c], in1=xs[c])
                    nc.tensor.matmul(Q[:], oi[:], x2[:],
                                     start=(c == 0), stop=(c == C - 1))
                m2 = ac.tile([1, T], f32, tag="lnst", bufs=6, name="lnm2")
                nc.scalar.activation(out=m2[:], in_=S[:], func=AF.Square)
                var = ac.tile([1, T], f32, tag="lnst", bufs=6, name="lnvar")
                nc.vector.tensor_sub(out=var[:], in0=Q[:], in1=m2[:])
                lnv = ac.tile([1, T], f32, tag="lnst", bufs=6, name="lnlnv")
                nc.scalar.activation(out=lnv[:], in_=var[:], func=AF.Ln,
                                     bias=epsc[0:1, :])
                pairb = ac.tile([1, 2 * T], bf16, tag="lnpr", bufs=3,
                                name=f"pair{tag}")
                rstd = pairb[:, 0:T]
                nc.scalar.activation(out=rstd, in_=lnv[:], func=AF.Exp,
                                     scale=-0.5)
                nc.vector.scalar_tensor_tensor(
                    out=pairb[:, T:2 * T], in0=S[:], scalar=-1.0,
                    in1=rstd, op0=OP.mult, op1=OP.mult)
                res = {"mursn": pairb[:, T:2 * T], "rstd": rstd}
                BR = pp.tile([128, T], f32, tag="st", bufs=2, name=f"BR{tag}")
                nc.tensor.matmul(BR[:], ones1[:], rstd, start=True, stop=True)
                rB = ac.tile([128, T], bf16, tag="lnrB", bufs=2, name="lnrB")
                nc.vector.tensor_copy(out=rB[:], in_=BR[:])
                res["rB"] = rB
                if want_mB:
                    BM = pp.tile([128, T], f32, tag="st", bufs=2,
                                 name=f"BM{tag}")
                    nc.tensor.matmul(BM[:], ones1[:], pairb[:, T:2 * T],
                                     start=True, stop=True)
                    mBn = ac.tile([128, T], bf16, tag="lnmB", bufs=2,
                                  name="lnmB")
                    nc.vector.tensor_copy(out=mBn[:], in_=BM[:])
                    res["mBn"] = mBn
                if want_col:
                    k = len(col_chunks)
                    pc = pp.tile([128, k], f32, tag="st", bufs=2,
                                 name=f"pc{tag}")
                    for i, (a, b) in enumerate(col_chunks):
                        m = b - a
                        nc.tensor.matmul(pc[0:m, i:i + 1],
                                         pairb[0:1, a:b], ones1[0:1, 0:1],
                                         start=True, stop=True)
                    rC = ac.tile([128, k], f32, tag="lnrC", bufs=2,
                                 name="lnrC")
                    nc.vector.tensor_copy(out=rC[:], in_=pc[:])
                    res["rC"] = rC
                return res

            def load_cols(dram_ap, n, tag, rows=128):
                ts = []
                for c in range(n):
                    t = wp.tile([rows, 1], f32, tag=f"{tag}{c}", bufs=1,
                                name=f"{tag}{c}")
                    nc.sync.dma_start(out=t[:],
                                      in_=dram_ap[rows * c:rows * (c + 1), :])
                    ts.append(t)
                return ts

            def ln_fm(xs, T, gs, bs, out_views):
                """Materialized feature-major layernorm (embed LN2 only)."""
                C = len(xs)
                inv = 1.0 / (128 * C)
                xbs = []
                S = pp.tile([1, T], f32, tag="g", name="lnS")
                Q = pp.tile([1, T], f32, tag="g", name="lnQ")
                for c in range(C):
                    xb = ac.tile([128, T], bf16, tag="lnxb", bufs=10,
                                 name="lnxb")
                    nc.vector.tensor_copy(out=xb[:], in_=xs[c])
                    xbs.append(xb)
                    nc.tensor.matmul(S[:], ones128[:], xb[:],
                                     start=(c == 0), stop=(c == C - 1))
                for c in range(C):
                    x2 = ac.tile([128, T], bf16, tag="lnx2", bufs=3,
                                 name="lnx2")
                    nc.vector.tensor_mul(out=x2[:], in0=xbs[c][:],
                                         in1=xbs[c][:])
                    nc.tensor.matmul(Q[:], ones128[:], x2[:],
                                     start=(c == 0), stop=(c == C - 1))
                mu = ac.tile([1, T], f32, tag="lnst", bufs=6, name="lnmu")
                nc.scalar.mul(out=mu[:], in_=S[:], mul=inv)
                m2 = ac.tile([1, T], f32, tag="lnst", bufs=6, name="lnm2")
                nc.vector.tensor_mul(out=m2[:], in0=mu[:], in1=mu[:])
                var = ac.tile([1, T], f32, tag="lnst", bufs=6, name="lnvar")
                nc.scalar.mul(out=var[:], in_=Q[:], mul=inv)
                nc.vector.tensor_sub(out=var[:], in0=var[:], in1=m2[:])
                rstd = ac.tile([1, T], f32, tag="lnst", bufs=6, name="lnrstd")
                nc.scalar.activation(out=rstd[:], in_=var[:], func=AF.Ln,
                                     bias=epsc[0:1, :])
                nc.scalar.activation(out=rstd[:], in_=rstd[:], func=AF.Exp,
                                     scale=-0.5)
                murs = ac.tile([1, T], f32, tag="lnst", bufs=6, name="lnmurs")
                nc.vector.tensor_mul(out=murs[:], in0=mu[:], in1=rstd[:])
                rb = ac.tile([1, T], bf16, tag="lnsb", bufs=4, name="lnrb")
                nc.vector.tensor_copy(out=rb[:], in_=rstd[:])
                mb = ac.tile([1, T], bf16, tag="lnsb", bufs=4, name="lnmb")
                nc.vector.tensor_copy(out=mb[:], in_=murs[:])
                BR = pp.tile([128, T], f32, tag="g", name="lnBR")
                nc.tensor.matmul(BR[:], ones1[:], rb[:], start=True,
                                 stop=True)
                BM = pp.tile([128, T], f32, tag="g", name="lnBM")
                nc.tensor.matmul(BM[:], ones1[:], mb[:], start=True,
                                 stop=True)
                rB = ac.tile([128, T], bf16, tag="lnrB", bufs=2, name="lnrB")
                nc.vector.tensor_copy(out=rB[:], in_=BR[:])
                mB = ac.tile([128, T], bf16, tag="lnmB", bufs=2, name="lnmB")
                nc.vector.tensor_copy(out=mB[:], in_=BM[:])
                for c in range(C):
                    t1 = ac.tile([128, T], bf16, tag="lnt1", bufs=2,
                                 name="lnt1")
                    nc.vector.tensor_mul(out=t1[:], in0=xbs[c][:], in1=rB[:])
                    nc.vector.tensor_sub(out=t1[:], in0=t1[:], in1=mB[:])
                    nc.vector.tensor_scalar(out=out_views[c], in0=t1[:],
                                            scalar1=gs[c][:],
                                            scalar2=bs[c][:], op0=OP.mult,
                                            op1=OP.add)

            tok_chunks = [(0, 128), (128, 256), (256, 384), (384, 400)]
            rg = [[0, 1, 2, 3], [4, 5, 6, 7]]

            # ---------- embed ----------
            xeT = ac.tile([128, 8, OWN], bf16, tag="xe", bufs=1, name="xeT")
            nc.sync.dma_start(out=xeT[:],
                              in_=x_t.rearrange("(c p) t -> p c t", c=8))
            xe = [xeT[:, c, :] for c in range(8)]
            ewT = wp.tile([128, 8, D], f8, tag="ew", bufs=1, name="ewT")
            nc.sync.dma_start(out=ewT[:],
                              in_=ew_t.rearrange("(c p) f -> p c f", c=8))
            ebs = load_cols(ebias_t, 4, "ebias")
            eg2s = load_cols(eg2, 4, "eg2")
            eb2s = load_cols(eb2, 4, "eb2")

            est = ln_stats(list(xe), OWN, 8, "emb", want_mB=True)
            xeq = [ac.tile([128, 2, OWN], f8, tag=f"xeq{c2}", bufs=1,
                           name=f"xeq{c2}") for c2 in range(4)]
            for c in range(8):
                tmp = ac.tile([128, OWN], bf16, tag="eyt", bufs=2, name="eyt")
                nc.gpsimd.tensor_mul(out=tmp[:], in0=xe[c], in1=est["rB"][:])
                nc.vector.tensor_add(out=xeq[c // 2][:, c % 2, :],
                                     in0=tmp[:], in1=est["mBn"][:])

            t2 = []
            for mc in range(4):
                ps = pp.tile([128, OWN], f32, tag="kv", bufs=4,
                             name=f"embp{mc}")
                for c2 in range(4):
                    nc.tensor.matmul(
                        ps[:], ewT[:, 2 * c2:2 * c2 + 2,
                                   128 * mc:128 * (mc + 1)],
                        xeq[c2][:], start=(c2 == 0), stop=(c2 == 3),
                        perf_mode=DR)
                t = ac.tile([128, OWN], f32, tag="t2", bufs=4, name=f"t2{mc}")
                nc.vector.tensor_scalar_add(out=t[:], in0=ps[:],
                                            scalar1=ebs[mc][:])
                t2.append(t[:])

            tok = [ac.tile([128, TOK], bf16, tag=f"tok{c}", bufs=1,
                           name=f"tok{c}") for c in range(4)]
            ln_fm(t2, OWN, eg2s, eb2s, [tok[c][:, 0:OWN] for c in range(4)])
            for c in range(4):
                nc.sync.dma_start(out=tok[c][:, OWN:TOK],
                                  in_=fus_t[128 * c:128 * (c + 1), :])

            def resid_and_cast(psO_prev):
                """tok += psO/256 (DVE c0/c1, Pool c2/c3), then fp8 cast."""
                if psO_prev is not None:
                    for c in range(4):
                        nc.vector.scalar_tensor_tensor(
                            out=tok[c][:], in0=psO_prev[c][:],
                            scalar=1.0 / 256, in1=tok[c][:],
                            op0=OP.mult, op1=OP.add)
                tq = [ac.tile([128, 2, TOK], f8, tag=f"tq{c2}", bufs=2,
                              name=f"tq{c2}") for c2 in range(2)]
                for c in range(4):
                    nc.vector.tensor_copy(out=tq[c // 2][:, c % 2, :],
                                          in_=tok[c][:])
                return tq

            # ---------- layers ----------
            psO_prev = None
            for l in range(DEPTH):
                wqT = wp.tile([128, 4, D], f8, tag="wq", bufs=1, name="wqT")
                nc.sync.dma_start(out=wqT[:],
                                  in_=wq_t[l].rearrange("(c p) f -> p c f",
                                                        c=4))
                wkvT = wp.tile([128, 4, 2 * D], f8, tag="wkv", bufs=2,
                               name="wkvT")
                nc.sync.dma_start(out=wkvT[:],
                                  in_=wkv_t[l].rearrange("(c p) f -> p c f",
                                                         c=4))
                wohT = wp.tile([DH, H, D], bf16, tag="woh", bufs=1,
                               name="wohT")
                nc.sync.dma_start(out=wohT[:],
                                  in_=woh_t[l].rearrange("h d f -> d h f"))
                wonT = wp.tile([128, 4, D], bf16, tag="won", bufs=1,
                               name="wonT")
                nc.sync.dma_start(out=wonT[:],
                                  in_=won_t[l].rearrange("(c p) f -> p c f",
                                                         c=4))
                won = [wonT[:, c, :] for c in range(4)]
                w1T = wp.tile([128, 4, 2 * IFFP], f8, tag="w1", bufs=1,
                              name="w1T")
                nc.sync.dma_start(out=w1T[:],
                                  in_=w1_t[l].rearrange("(c p) f -> p c f",
                                                        c=4))
                w2T_ = wp.tile([128, W2T, D], f8, tag="w2", bufs=1,
                               name="w2T_")
                nc.sync.dma_start(out=w2T_[:],
                                  in_=w2_t[l].rearrange("(j p) f -> p j f",
                                                        j=W2T))
                wbk = wp.tile([1, D], bf16, tag="wbk", bufs=2, name="wbk")
                nc.sync.dma_start(out=wbk[:], in_=wbar_t[l, 0:1, :])
                wbv = wp.tile([1, D], bf16, tag="wbv", bufs=2, name="wbv")
                nc.sync.dma_start(out=wbv[:], in_=wbar_t[l, 1:2, :])
                wbq = wp.tile([1, D], bf16, tag="wbq", bufs=2, name="wbq")
                nc.sync.dma_start(out=wbq[:], in_=wbar_t[l, 2:3, :])

                tokq = resid_and_cast(psO_prev)
                psO_prev = None

                # raw K^T / q^T projections (start immediately off tokq)
                ktp = []
                for mc in range(4):
                    ps = pp.tile([128, OWN], f32, tag="kv", bufs=4,
                                 name=f"kt{mc}")
                    for c2 in range(2):
                        nc.tensor.matmul(
                            ps[:], wkvT[:, 2 * c2:2 * c2 + 2,
                                        128 * mc:128 * (mc + 1)],
                            tokq[c2][:, :, 0:OWN],
                            start=(c2 == 0), stop=False, perf_mode=DR)
                    ktp.append(ps)
                qfp = pp.tile([128, 4, FUS], f32, tag="g", name="qfp")
                for mc in range(4):
                    for c2 in range(2):
                        nc.tensor.matmul(
                            qfp[:, mc, :], wqT[:, 2 * c2:2 * c2 + 2,
                                               128 * mc:128 * (mc + 1)],
                            tokq[c2][:, :, OWN:TOK],
                            start=(c2 == 0), stop=False, perf_mode=DR)

                # LN1 stats run concurrently with the raw projections
                st1 = ln_stats([tok[c][:] for c in range(4)], TOK, 4,
                               f"l1{l}", want_col=True,
                               col_chunks=tok_chunks)

                # corrections + rstd applies
                kt = []
                for mc in range(4):
                    nc.tensor.matmul(ktp[mc][:],
                                     wbk[:, 128 * mc:128 * (mc + 1)],
                                     st1["mursn"][:, 0:OWN],
                                     start=False, stop=True)
                    s = ac.tile([128, OWN], bf16, tag=f"kt{mc}", bufs=1,
                                name=f"ktb{mc}")
                    nc.vector.tensor_mul(out=s[:], in0=ktp[mc][:],
                                         in1=st1["rB"][:, 0:OWN])
                    kt.append(s)
                qf = []
                for mc in range(4):
                    nc.tensor.matmul(qfp[:, mc, :],
                                     wbq[:, 128 * mc:128 * (mc + 1)],
                                     st1["mursn"][:, OWN:TOK],
                                     start=False, stop=True)
                    s = ac.tile([128, 32], bf16, tag=f"qf{mc}", bufs=1,
                                name=f"qfb{mc}")
                    nc.vector.memset(s[:, FUS:32], 0.0)
                    nc.vector.tensor_mul(out=s[:, 0:FUS], in0=qfp[:, mc, :],
                                         in1=st1["rB"][:, OWN:TOK])
                    qf.append(s)

                # V token-major raw + correction, rstd col-scale on copy
                V = []
                for i, (a, b) in enumerate(tok_chunks):
                    m = b - a
                    ps = pp.tile([128, D], f32, tag="kv", bufs=4,
                                 name=f"v{i}")
                    for c2 in range(2):
                        nc.tensor.matmul(ps[0:m, :],
                                         tokq[c2][:, :, a:b],
                                         wkvT[:, 2 * c2:2 * c2 + 2, D:2 * D],
                                         start=(c2 == 0), stop=False,
                                         perf_mode=DR)
                    nc.tensor.matmul(ps[0:m, :], st1["mursn"][:, a:b],
                                     wbv[:], start=False, stop=True)
                    s = ac.tile([128, D], bf16, tag=f"V{i}", bufs=1,
                                name=f"Vb{i}")
                    nc.scalar.activation(out=s[0:m, :], in_=ps[0:m, :],
                                         func=AF.Copy,
                                         scale=st1["rC"][0:m, i:i + 1])
                    V.append(s)

                # V column sums: own (exchanged early) + fusion (local)
                vsf = pp.tile([128, 8], f32, tag="g", name="vsf")
                for c in range(4):
                    for j in range(3):
                        nc.tensor.matmul(vsf[:, c:c + 1],
                                         V[j][:, 128 * c:128 * (c + 1)],
                                         ones128[:], start=(j == 0),
                                         stop=(j == 2))
                for c in range(4):
                    nc.tensor.matmul(vsf[:, 4 + c:5 + c],
                                     V[3][0:FUS, 128 * c:128 * (c + 1)],
                                     ones128[0:FUS, :], start=True, stop=True)
                Pv = ac.tile([128, 4], f32, tag="Pv", bufs=2, name="Pv")
                nc.vector.tensor_copy(out=Pv[:], in_=vsf[:, 0:4])
                vfu = ac.tile([128, 4], f32, tag="vfu", bufs=2, name="vfu")
                nc.vector.tensor_copy(out=vfu[:], in_=vsf[:, 4:8])
                pin1 = dramp.tile([128, 4], f32, tag="pin1", bufs=2,
                                  name="pin1")
                nc.sync.dma_start(out=pin1[:], in_=Pv[:])
                Rv = ac.tile([128, 4, 4], f32, tag="Rv", bufs=2, name="Rv")
                if use_cc:
                    pout1 = dramp.tile([4 * 128, 4], f32, tag="pout1",
                                       bufs=2, name="pout1")
                    nc.gpsimd.collective_compute(
                        "AllGather", OP.bypass, replica_groups=rg,
                        ins=[pin1.opt()], outs=[pout1.opt()])
                    nc.sync.dma_start(
                        out=Rv[:],
                        in_=pout1.rearrange("(r p) f -> p r f", r=4))
                else:
                    nc.sync.dma_start(
                        out=Rv[:],
                        in_=pin1.rearrange("(r p) f -> p r f", r=1)
                        .to_broadcast((128, 4, 4)))

                # scores + exp (+row sums); kt/qf carry 2^16 scale
                E, lacc = [], []
                for t in range(2):
                    sp = pp.tile([128, OWN], f32, tag="g", name=f"sp{t}")
                    for i in range(4):
                        h = 4 * t + i
                        ch, base = h // 2, (h % 2) * 64
                        nc.tensor.matmul(sp[32 * i:32 * i + 32, :],
                                         qf[ch][base:base + 64, 0:32],
                                         kt[ch][base:base + 64, :],
                                         start=True, stop=True,
                                         tile_position=(base, 32 * i))
                    e = ac.tile([128, OWN], bf16, tag=f"e{t}", bufs=1,
                                name=f"e{t}")
                    la = ac.tile([128, 1], f32, tag=f"la{t}", bufs=2,
                                 name=f"la{t}")
                    nc.scalar.activation(out=e[:], in_=sp[:], func=AF.Exp,
                                         scale=1.0 / 65536, accum_out=la[:])
                    E.append(e)
                    lacc.append(la)

                ET = [[None] * 3 for _ in range(2)]
                for t in range(2):
                    for j in range(3):
                        pt = pp.tile([128, 128], bf16, tag="g",
                                     name=f"et{t}{j}")
                        nc.tensor.transpose(pt[:],
                                            E[t][:, 128 * j:128 * (j + 1)],
                                            ident[:])
                        s = ac.tile([128, 128], bf16, tag=f"ET{t}{j}",
                                    bufs=1, name=f"ETb{t}{j}")
                        nc.vector.tensor_copy(out=s[:], in_=pt[:])
                        ET[t][j] = s

                # payload P = [l0, l1, ACC0, ACC1]
                P = ac.tile([128, 130], f32, tag="P", bufs=2, name="P")
                nc.vector.tensor_copy(out=P[:, 0:1], in_=lacc[0][:])
                nc.vector.tensor_copy(out=P[:, 1:2], in_=lacc[1][:])
                accp = pp.tile([128, 2, 64], f32, tag="g", name="accp")
                for t in range(2):
                    for i in range(4):
                        h = 4 * t + i
                        for j in range(3):
                            nc.tensor.matmul(accp[32 * i:32 * i + 32, t, :],
                                             ET[t][j][:, 32 * i:32 * i + 32],
                                             V[j][:, DH * h:DH * (h + 1)],
                                             start=(j == 0), stop=(j == 2),
                                             tile_position=(0, 32 * i))
                    nc.vector.tensor_copy(out=P[:, 2 + 64 * t:66 + 64 * t],
                                          in_=accp[:, t, :])
                pin2 = dramp.tile([128, 130], f32, tag="pin2", bufs=2,
                                  name="pin2")
                nc.sync.dma_start(out=pin2[:], in_=P[:])
                R2 = ac.tile([128, 4, 130], f32, tag="R2", bufs=2, name="R2")
                if use_cc:
                    pout2 = dramp.tile([4 * 128, 130], f32, tag="pout2",
                                       bufs=2, name="pout2")
                    nc.gpsimd.collective_compute(
                        "AllGather", OP.bypass, replica_groups=rg,
                        ins=[pin2.opt()], outs=[pout2.opt()])
                    nc.sync.dma_start(
                        out=R2[:],
                        in_=pout2.rearrange("(r p) f -> p r f", r=4))
                else:
                    nc.sync.dma_start(
                        out=R2[:],
                        in_=pin2.rearrange("(r p) f -> p r f", r=1)
                        .to_broadcast((128, 4, 130)))

                # uniform delta from the early vsum exchange
                vT2 = ac.tile([128, 2, 4], f32, tag="cmb", bufs=3,
                              name="vT2")
                nc.vector.tensor_add(out=vT2[:], in0=Rv[:, 0:2, :],
                                     in1=Rv[:, 2:4, :])
                vsb = ac.tile([128, 4], bf16, tag="vsb", bufs=2, name="vsb")
                nc.vector.scalar_tensor_tensor(
                    out=vsb[:], in0=vT2[:, 0, :], scalar=1.0,
                    in1=vT2[:, 1, :], op0=OP.bypass, op1=OP.add)
                nc.vector.tensor_add(out=vsb[:], in0=vsb[:], in1=vfu[:])
                dup = pp.tile([128, 4], f32, tag="g", name="dup")
                for c in range(4):
                    for kc in range(4):
                        nc.tensor.matmul(dup[:, c:c + 1],
                                         won[kc][:, 128 * c:128 * (c + 1)],
                                         vsb[:, kc:kc + 1],
                                         start=(kc == 0), stop=(kc == 3))
                dub = ac.tile([128, 4], f32, tag="dub", bufs=2, name="dub")
                nc.vector.tensor_copy(out=dub[:], in_=dup[:])
                dus = [dub[:, c:c + 1] for c in range(4)]

                # fusion delta, directly feature-major [128, 4c, 16]
                T01 = ac.tile([128, 130], f32, tag="cmb2", bufs=3,
                              name="T01")
                nc.vector.tensor_add(out=T01[:], in0=R2[:, 0, :],
                                     in1=R2[:, 1, :])
                T23 = ac.tile([128, 130], f32, tag="cmb2", bufs=3,
                              name="T23")
                nc.vector.tensor_add(out=T23[:], in0=R2[:, 2, :],
                                     in1=R2[:, 3, :])
                PT = ac.tile([128, 130], f32, tag="cmb2", bufs=3, name="PT")
                nc.vector.tensor_add(out=PT[:], in0=T01[:], in1=T23[:])
                linv = ac.tile([128, 2], f32, tag="linv", bufs=2,
                               name="linv")
                nc.vector.reciprocal(out=linv[:], in_=PT[:, 0:2])
                ofT = []
                for t in range(2):
                    of = ac.tile([128, 64], bf16, tag=f"of{t}", bufs=1,
                                 name=f"of{t}")
                    nc.vector.tensor_scalar_mul(
                        out=of[:], in0=PT[:, 2 + 64 * t:66 + 64 * t],
                        scalar1=linv[:, t:t + 1])
                    pt = pp.tile([64, 128], bf16, tag="g", name=f"oft{t}")
                    nc.tensor.transpose(pt[:], of[:], ident[:])
                    s = ac.tile([64, 128], bf16, tag=f"ofT{t}", bufs=1,
                                name=f"ofTb{t}")
                    nc.vector.tensor_copy(out=s[:], in_=pt[:])
                    ofT.append(s)
                dfT = pp.tile([128, 4, FUS], f32, tag="g", name="dfT")
                for h in range(H):
                    t, i = h // 4, h % 4
                    for c in range(4):
                        nc.tensor.matmul(
                            dfT[:, c, :],
                            wohT[:, h, 128 * c:128 * (c + 1)],
                            ofT[t][:, 32 * i:32 * i + FUS],
                            start=(h == 0), stop=(h == H - 1))

                # LN2: residual applied in place, then stats + fp8 y
                for c in range(4):
                    nc.vector.tensor_scalar_add(out=tok[c][:, 0:OWN],
                                                in0=tok[c][:, 0:OWN],
                                                scalar1=dus[c])
                    nc.vector.tensor_add(out=tok[c][:, OWN:TOK],
                                         in0=tok[c][:, OWN:TOK],
                                         in1=dfT[:, c, :])
                st2 = ln_stats([tok[c][:] for c in range(4)], TOK, 4,
                               f"l2{l}", want_mB=True)
                xn2q = [ac.tile([128, 2, TOK], f8, tag=f"x2q{c2}", bufs=1,
                                name=f"x2q{c2}") for c2 in range(2)]
                for c in range(4):
                    tmp = ac.tile([128, TOK], bf16, tag="yt2", bufs=2,
                                  name="yt2")
                    nc.gpsimd.tensor_mul(out=tmp[:], in0=tok[c][:],
                                         in1=st2["rB"][:])
                    nc.vector.tensor_add(out=xn2q[c // 2][:, c % 2, :],
                                         in0=tmp[:], in1=st2["mBn"][:])

                # GEGLU FF in fp8 DoubleRow
                gtq = [ac.tile([128, 2, TOK], f8, tag=f"gtq{jj}", bufs=1,
                               name=f"gtq{jj}") for jj in range(5)]
                gt10 = ac.tile([128, TOK], f8, tag="gt10", bufs=1,
                               name="gt10")
                for j in range(W2T):
                    a = 128 * j
                    px = pp.tile([128, TOK], f32, tag="g", name=f"fx{j}")
                    pg = pp.tile([128, TOK], f32, tag="g", name=f"fg{j}")
                    for c2 in range(2):
                        nc.tensor.matmul(
                            px[:], w1T[:, 2 * c2:2 * c2 + 2, a:a + 128],
                            xn2q[c2][:], start=(c2 == 0), stop=(c2 == 1),
                            perf_mode=DR)
                    for c2 in range(2):
                        nc.tensor.matmul(
                            pg[:], w1T[:, 2 * c2:2 * c2 + 2,
                                       IFFP + a:IFFP + a + 128],
                            xn2q[c2][:], start=(c2 == 0), stop=(c2 == 1),
                            perf_mode=DR)
                    gg = ac.tile([128, TOK], bf16, tag="gg", bufs=3,
                                 name=f"gg{j}")
                    nc.scalar.activation(out=gg[:], in_=pg[:], func=AF.Gelu,
                                         scale=1.0 / 128)
                    gdst = gtq[j // 2][:, j % 2, :] if j < 10 else gt10[:]
                    nc.vector.tensor_mul(out=gdst, in0=gg[:], in1=px[:])
                psO_prev = []
                for c in range(4):
                    psO = pp.tile([128, TOK], f32, tag="kv", bufs=4,
                                  name=f"fo{c}")
                    for jj in range(5):
                        nc.tensor.matmul(
                            psO[:], w2T_[:, 2 * jj:2 * jj + 2,
                                         128 * c:128 * (c + 1)],
                            gtq[jj][:], start=(jj == 0), stop=False,
                            perf_mode=DR)
                    nc.tensor.matmul(psO[:],
                                     w2T_[:, 10, 128 * c:128 * (c + 1)],
                                     gt10[:], start=False, stop=True)
                    psO_prev.append(psO)

            # ---------- pool ----------
            pwkvT = wp.tile([128, 4, 2 * D], f8, tag="pwkv", bufs=1,
                            name="pwkvT")
            nc.sync.dma_start(out=pwkvT[:],
                              in_=pwkv_t.rearrange("(c p) f -> p c f", c=4))
            pbk = wp.tile([1, D], bf16, tag="pbk", bufs=1, name="pbk")
            nc.sync.dma_start(out=pbk[:], in_=pbar_t[0:1, :])
            pbv = wp.tile([1, D], bf16, tag="pbv", bufs=1, name="pbv")
            nc.sync.dma_start(out=pbv[:], in_=pbar_t[1:2, :])
            pwoh = []
            for h in range(H):
                t = wp.tile([DH, D], bf16, tag=f"woh{h}", bufs=1,
                            name=f"pwoh{h}")
                nc.sync.dma_start(out=t[:], in_=pwoh_t[h])
                pwoh.append(t)
            pwon = []
            for c in range(4):
                t = wp.tile([128, D], bf16, tag=f"pwon{c}", bufs=1,
                            name=f"pwon{c}")
                nc.sync.dma_start(out=t[:],
                                  in_=pwon_t[128 * c:128 * (c + 1), :])
                pwon.append(t)
            pq2s = load_cols(pq2_t, 4, "pq2")

            tokq = resid_and_cast(psO_prev)
            stp = ln_stats([tok[c][:] for c in range(4)], TOK, 4, "pool",
                           want_col=True, col_chunks=tok_chunks)

            # V_pool token-major
            Vp = []
            for i, (a, b) in enumerate(tok_chunks):
                m = b - a
                ps = pp.tile([128, D], f32, tag="kv", bufs=4, name=f"pv{i}")
                for c2 in range(2):
                    nc.tensor.matmul(ps[0:m, :], tokq[c2][:, :, a:b],
                                     pwkvT[:, 2 * c2:2 * c2 + 2, D:2 * D],
                                     start=(c2 == 0), stop=False,
                                     perf_mode=DR)
                nc.tensor.matmul(ps[0:m, :], stp["mursn"][:, a:b], pbv[:],
                                 start=False, stop=True)
                s = ac.tile([128, D], bf16, tag=f"V{i}", bufs=1,
                            name=f"pVb{i}")
                nc.scalar.activation(out=s[0:m, :], in_=ps[0:m, :],
                                     func=AF.Copy,
                                     scale=stp["rC"][0:m, i:i + 1])
                Vp.append(s)

            # pool vsum exchange (Vp carries 256x scale; pwon descales)
            pvsf = pp.tile([128, 8], f32, tag="g", name="pvsf")
            for c in range(4):
                for j in range(3):
                    nc.tensor.matmul(pvsf[:, c:c + 1],
                                     Vp[j][:, 128 * c:128 * (c + 1)],
                                     ones128[:], start=(j == 0),
                                     stop=(j == 2))
            for c in range(4):
                nc.tensor.matmul(pvsf[:, 4 + c:5 + c],
                                 Vp[3][0:FUS, 128 * c:128 * (c + 1)],
                                 ones128[0:FUS, :], start=True, stop=True)
            Pp = ac.tile([128, 4], f32, tag="Pp", bufs=2, name="Pp")
            nc.vector.tensor_copy(out=Pp[:], in_=pvsf[:, 0:4])
            pvfu = ac.tile([128, 4], f32, tag="vfu", bufs=2, name="pvfu")
            nc.vector.tensor_copy(out=pvfu[:], in_=pvsf[:, 4:8])
            pinp = dramp.tile([128, 4], f32, tag="pinp", bufs=1, name="pinp")
            nc.sync.dma_start(out=pinp[:], in_=Pp[:])
            Rpa = ac.tile([128, 4, 4], f32, tag="Rv", bufs=2, name="Rpa")
            if use_cc:
                poutp = dramp.tile([4 * 128, 4], f32, tag="poutp", bufs=1,
                                   name="poutp")
                nc.gpsimd.collective_compute(
                    "AllGather", OP.bypass, replica_groups=rg,
                    ins=[pinp.opt()], outs=[poutp.opt()])
                nc.sync.dma_start(
                    out=Rpa[:],
                    in_=poutp.rearrange("(r p) f -> p r f", r=4))
            else:
                nc.sync.dma_start(
                    out=Rpa[:],
                    in_=pinp.rearrange("(r p) f -> p r f", r=1)
                    .to_broadcast((128, 4, 4)))
            pT2 = ac.tile([128, 2, 4], f32, tag="cmb", bufs=3, name="pT2")
            nc.vector.tensor_add(out=pT2[:], in0=Rpa[:, 0:2, :],
                                 in1=Rpa[:, 2:4, :])
            pvsb = ac.tile([128, 4], bf16, tag="vsb", bufs=2, name="pvsb")
            nc.vector.scalar_tensor_tensor(
                out=pvsb[:], in0=pT2[:, 0, :], scalar=1.0,
                in1=pT2[:, 1, :], op0=OP.bypass, op1=OP.add)
            nc.vector.tensor_add(out=pvsb[:], in0=pvsb[:], in1=pvfu[:])
            pdup = pp.tile([128, 4], f32, tag="g", name="pdup")
            for c in range(4):
                for kc in range(4):
                    nc.tensor.matmul(pdup[:, c:c + 1],
                                     pwon[kc][:, 128 * c:128 * (c + 1)],
                                     pvsb[:, kc:kc + 1],
                                     start=(kc == 0), stop=(kc == 3))
            pdub = ac.tile([128, 4], f32, tag="du", bufs=2, name="pdub")
            nc.vector.tensor_copy(out=pdub[:], in_=pdup[:])
            for c in range(4):
                nc.sync.dma_start(out=out_u[128 * c:128 * (c + 1), :],
                                  in_=pdub[:, c:c + 1])

            # fusion-key attention for return token 2 (all local)
            kf = []
            kfp = pp.tile([128, 6, FUS], f32, tag="g", name="kfp")
            for mc in range(4):
                for c2 in range(2):
                    nc.tensor.matmul(kfp[:, mc, :],
                                     pwkvT[:, 2 * c2:2 * c2 + 2,
                                           128 * mc:128 * (mc + 1)],
                                     tokq[c2][:, :, OWN:TOK],
                                     start=(c2 == 0), stop=False,
                                     perf_mode=DR)
                nc.tensor.matmul(kfp[:, mc, :],
                                 pbk[:, 128 * mc:128 * (mc + 1)],
                                 stp["mursn"][:, OWN:TOK],
                                 start=False, stop=True)
                s = ac.tile([128, FUS], bf16, tag=f"kf{mc}", bufs=1,
                            name=f"kfb{mc}")
                nc.vector.tensor_mul(out=s[:], in0=kfp[:, mc, :],
                                     in1=stp["rB"][:, OWN:TOK])
                kf.append(s)
            q2 = []
            for mc in range(4):
                s = ac.tile([128, 32], bf16, tag=f"q2{mc}", bufs=1,
                            name=f"q2b{mc}")
                nc.vector.memset(s[:, 1:32], 0.0)
                nc.vector.tensor_copy(out=s[:, 0:1], in_=pq2s[mc][:])
                q2.append(s)
            e2, l2 = [], []
            for t in range(2):
                sp = kfp[:, 4 + t, :]
                for i in range(4):
                    h = 4 * t + i
                    ch, base = h // 2, (h % 2) * 64
                    nc.tensor.matmul(sp[32 * i:32 * i + 32, :],
                                     q2[ch][base:base + 64, 0:32],
                                     kf[ch][base:base + 64, :],
                                     start=True, stop=True,
                                     tile_position=(base, 32 * i))
                e = ac.tile([128, FUS], bf16, tag=f"e2{t}", bufs=1,
                            name=f"e2{t}")
                la = ac.tile([128, 1], f32, tag=f"la{t}", bufs=2,
                             name=f"pla{t}")
                nc.scalar.activation(out=e[:], in_=sp[:], func=AF.Exp,
                                     scale=1.0 / 256, accum_out=la[:])
                e2.append(e)
                l2.append(la)
            e2T = []
            for t in range(2):
                pt = pp.tile([FUS, 128], bf16, tag="g", name=f"pet{t}")
                nc.tensor.transpose(pt[:], e2[t][:], ident[:])
                s = ac.tile([FUS, 128], bf16, tag=f"e2T{t}", bufs=1,
                            name=f"e2Tb{t}")
                nc.vector.tensor_copy(out=s[:], in_=pt[:])
                e2T.append(s)
            ofT2 = []
            pacc2 = pp.tile([128, 2, 64], f32, tag="g", name="pacc2")
            for t in range(2):
                for i in range(4):
                    h = 4 * t + i
                    nc.tensor.matmul(pacc2[32 * i:32 * i + 32, t, :],
                                     e2T[t][:, 32 * i:32 * i + 32],
                                     Vp[3][0:FUS, DH * h:DH * (h + 1)],
                                     start=True, stop=True,
                                     tile_position=(0, 32 * i))
                li = ac.tile([128, 1], f32, tag="linv", bufs=2,
                             name=f"pli{t}")
                nc.vector.reciprocal(out=li[:], in_=l2[t][:])
                of = ac.tile([128, 64], bf16, tag=f"of{t}", bufs=1,
                             name=f"pof{t}")
                nc.vector.tensor_scalar_mul(out=of[:], in0=pacc2[:, t, :],
                                            scalar1=li[:])
                pt = pp.tile([64, 128], bf16, tag="g", name=f"poft{t}")
                nc.tensor.transpose(pt[:], of[:], ident[:])
                s = ac.tile([64, 128], bf16, tag=f"ofT{t}", bufs=1,
                            name=f"pofTb{t}")
                nc.vector.tensor_copy(out=s[:], in_=pt[:])
                ofT2.append(s)
            # P2^T feature-major [128, 4]: 32 matmuls moving 1
            P2 = pp.tile([128, 4], f32, tag="g", name="P2")
            for h in range(H):
                t, i = h // 4, h % 4
                for c in range(4):
                    nc.tensor.matmul(P2[:, c:c + 1],
                                     pwoh[h][:, 128 * c:128 * (c + 1)],
                                     ofT2[t][:, 32 * i:32 * i + 1],
                                     start=(h == 0), stop=(h == H - 1))
            p2s = ac.tile([128, 4], f32, tag="p2s", bufs=1, name="p2s")
            nc.vector.tensor_copy(out=p2s[:], in_=P2[:])
            nc.sync.dma_start(out=out_f[:], in_=p2s[:])

    nc.compile()
    _built[key] = nc
    return nc


def _pad_w1(w1f):
    """[DEPTH, D, 2*IFF] -> [DEPTH, D, 2*1408] with x1/gate blocks padded."""
    out = np.zeros((DEPTH, D, 2 * 1408), np.float64)
    out[:, :, 0:IFF] = w1f[:, :, 0:IFF]
    out[:, :, 1408:1408 + IFF] = w1f[:, :, IFF:2 * IFF]
    return out


def _q8(x, s):
    """fp8e4m3 quantize with a power-of-2 scale folded in."""
    return np.clip(np.asarray(x, np.float64) * s, -224, 224).astype(F8)


def _prep_inputs(inputs):
    """Host-side prep: slice/transpose/cast/quantize per-core input dicts."""
    I = {k: np.asarray(v) for k, v in inputs.items()}
    f32 = np.float32

    def bf(x):
        return np.ascontiguousarray(x).astype(BF)

    def col(x):
        return np.ascontiguousarray(np.asarray(x, f32).reshape(-1, 1))

    scale = DH ** -0.5
    wqf = I["layers_wq"].astype(np.float64) * scale \
        * I["layers_attn_g"].astype(np.float64)[:, :, None]
    wkvf = I["layers_wkv"].astype(np.float64) \
        * I["layers_attn_g"].astype(np.float64)[:, :, None]
    w1f = _pad_w1(I["layers_ff_w1"].astype(np.float64)
                  * I["layers_ff_g"].astype(np.float64)[:, :, None])
    w1f[:, :, 0:1408] *= 8.0       # x1 half
    w1f[:, :, 1408:] *= 128.0      # gate half
    w2f = np.pad(I["layers_ff_w2"].astype(np.float64),
                 ((0, 0), (0, 1408 - IFF), (0, 0))) * 32.0
    pkvf = I["pool_wkv"].astype(np.float64) \
        * I["final_g"].astype(np.float64)[:, None]

    wq_q = _q8(wqf, 256.0)
    wkv_q = _q8(wkvf, 256.0)
    w1_q = np.clip(w1f, -224, 224).astype(F8)
    w2_q = np.clip(w2f, -224, 224).astype(F8)
    pkv_q = _q8(pkvf, 256.0)

    wkv_d = wkv_q.astype(np.float64)
    wq_d = wq_q.astype(np.float64)
    pkv_d = pkv_q.astype(np.float64)
    wbar = np.stack([wkv_d[:, :, 0:D].sum(axis=1),       # k-half
                     wkv_d[:, :, D:2 * D].sum(axis=1),   # v-half
                     wq_d.sum(axis=1)], axis=1)          # q
    pbar = np.stack([pkv_d[:, 0:D].sum(axis=0),
                     pkv_d[:, D:2 * D].sum(axis=0)], axis=0)

    shared = {
        "fus_t": bf(I["fusion_tokens"].astype(np.float64).T),
        "wq": wq_q,
        "wkv": wkv_q,
        "wo_h": bf(I["layers_wo"].reshape(DEPTH, H, DH, D) / 256.0),
        "wo_n": bf(I["layers_wo"] * (1.0 / (NALL * 256.0))),
        "w1": w1_q,
        "w2": w2_q,
        "wbar": bf(wbar),
        "pool_wkv": pkv_q,
        "pool_wbar": bf(pbar),
        "pool_wo_h": bf(I["pool_wo"].reshape(H, DH, D) / 256.0),
        "pool_wo_n": bf(I["pool_wo"] * (1.0 / (NALL * 256.0))),
    }
    # host-side pool query for return token 2 (row 2 = FUSION)
    ret = I["return_tokens"].astype(f32)
    g = I["pool_g"].astype(f32)
    mu = ret.mean(-1, keepdims=True)
    var = ((ret - mu) ** 2).mean(-1, keepdims=True)
    retn = (ret - mu) / np.sqrt(var + 1e-5) * g
    q2 = (retn[2] @ I["pool_wq"].astype(f32)) * scale
    shared["pool_q2"] = col(q2)

    in_maps = []
    for c in range(N_CORES):
        b, q = c // 4, c % 4
        mod = "rna" if q < 2 else "atac"
        x = I[mod][b, (q % 2) * OWN:(q % 2 + 1) * OWN, :]  # [384, 1024]
        m = dict(shared)
        m["x_t"] = bf(x.astype(np.float64).T)
        ewf = I[f"{mod}_w"].astype(np.float64) \
            * I[f"{mod}_ln1_g"].astype(np.float64)[:, None]
        m["emb_w"] = _q8(ewf, 256.0)
        m["emb_b"] = col((I[f"{mod}_b"].astype(np.float64)
                          + I[f"{mod}_ln1_b"].astype(np.float64)
                          @ I[f"{mod}_w"].astype(np.float64)) * 256.0)
        m["eln2_g"] = col(I[f"{mod}_ln2_g"])
        m["eln2_b"] = col(I[f"{mod}_ln2_b"])
        in_maps.append(m)
    return in_maps, ret


def kernel(**inputs):
    from concourse import bass_utils
    nc = build(num_devices=N_CORES, use_cc=True)
    in_maps, ret = _prep_inputs(inputs)
    res = bass_utils.run_bass_kernel_spmd(nc, in_maps,
                                          core_ids=list(range(N_CORES)))
    out = np.zeros((B, 3, D), np.float32)
    for b in range(2):
        r = res.results[4 * b]
        u = r["out_u"][:, 0]
        f = r["out_f"].T.ravel()
        out[b, 0] = u + ret[0]
        out[b, 1] = u + ret[1]
        out[b, 2] = f + ret[2]
    return out


# revision 34
# speedup vs baseline: 1.2281x; 1.2281x over previous
"""BioZorro sparse-attention kernel for 8 Trainium2 NeuronCores.

Sharding: 8 cores = 2 batches x 4 token-quarters (384 own tokens each).
The zorro mask makes all non-fusion query rows fully masked -> uniform
softmax -> their attention output is mean(V) over all tokens; only the 16
fusion tokens do real attention (over the 1536 non-fusion keys). Fusion
rows are replicated on all 4 cores of a batch; cross-core data is a tiny
per-layer AllGather of V column sums (early) plus one of flash-softmax
partials, and one small AllGather for the final pooling.

Perf structure vs the bf16 baseline:
- Heavy matmuls run fp8e4m3 with DoubleRow perf mode (two 128-row
  contraction tiles per instruction). Scales are powers of two folded
  into host weight prep (wq/wkv/ew/pwkv x256, w1 gate x128 / x1 x8,
  w2 x32), undone for free via exp/gelu activation scales and one
  scalar_tensor_tensor residual add (psO/256 + tok).
- LNs feeding attention projections are DEFERRED: projections consume
  the raw fp8 residual stream immediately; the -mu*rstd rank-1 mean
  correction lands in PSUM via one extra matmul per tile (lhsT = host
  column sums of the quantized weights), and rstd is applied during the
  PSUM->SBUF copy. The stats chain runs concurrently on ACT/DVE/Pool.
- wo folded: wo_h x(1/256) fusion path, wo_n x(1/(N*256)) mean-V path.
- PSUM rings: kv(4x2KB: kt/V/psO share), st(2x1.6K stats), g(2x1.6K),
  qf(4x64B), acc(2x256B), dfT(1x256B) -- 15.5KB of the 16KB budget.
"""
import sys
sys.path.insert(0, "/opt/trn_rl_repo")
import numpy as np
import ml_dtypes

BF = ml_dtypes.bfloat16
F8 = ml_dtypes.float8_e4m3
OWN, FUS, TOK = 384, 16, 400
D, RIN, H, DH, IFF, DEPTH = 512, 1024, 8, 64, 1365, 4
NALL = 1552
B, NR, NA = 2, 768, 768
N_CORES = 8

_built = {}


def build(num_devices=8, use_cc=True):
    key = (num_devices, use_cc)
    if key in _built:
        return _built[key]
    import concourse.tile as tile
    from concourse import bacc, mybir
    from concourse.masks import make_identity

    # Keep Exp resolvable from natural_log_exp_and_others so the Ln/Exp
    # pairs in the LN rstd chain share one ACT table set.
    if not getattr(bacc, "_act_tables_patched", False):
        _orig_gat = bacc.get_activation_tables

        def _patched_gat(arch):
            tabs = _orig_gat(arch)
            exp_t = mybir.ActivationFunctionType.Exp
            ln_t = mybir.ActivationFunctionType.Ln
            for nm, fns in tabs.items():
                if nm != "natural_log_exp_and_others":
                    fns.discard(exp_t)
                    fns.discard(ln_t)
            return tabs

        bacc.get_activation_tables = _patched_gat
        bacc._act_tables_patched = True

    f32 = mybir.dt.float32
    bf16 = mybir.dt.bfloat16
    f8 = mybir.dt.float8e4
    AF = mybir.ActivationFunctionType
    OP = mybir.AluOpType
    DR = mybir.MatmulPerfMode.DoubleRow

    nc = bacc.Bacc("TRN2", target_bir_lowering=False, debug=False,
                   enable_asserts=True, num_devices=num_devices)

    def din(name, shape, dt=f32):
        return nc.dram_tensor(name, shape, dt, kind="ExternalInput").ap()

    x_t = din("x_t", [RIN, OWN], bf16)
    ew_t = din("emb_w", [RIN, D], f8); ebias_t = din("emb_b", [D, 1])
    eg2 = din("eln2_g", [D, 1]); eb2 = din("eln2_b", [D, 1])
    fus_t = din("fus_t", [D, FUS], bf16)

    wq_t = din("wq", [DEPTH, D, D], f8)
    wkv_t = din("wkv", [DEPTH, D, 2 * D], f8)
    woh_t = din("wo_h", [DEPTH, H, DH, D], bf16)
    won_t = din("wo_n", [DEPTH, D, D], bf16)
    w1_t = din("w1", [DEPTH, D, 2 * 1408], f8)
    w2_t = din("w2", [DEPTH, 1408, D], f8)
    wbar_t = din("wbar", [DEPTH, 3, D], bf16)  # rows: kv_k, kv_v, q colsums
    pq2_t = din("pool_q2", [D, 1])
    pwkv_t = din("pool_wkv", [D, 2 * D], f8)
    pbar_t = din("pool_wbar", [2, D], bf16)  # rows: k-half, v-half colsums
    pwoh_t = din("pool_wo_h", [H, DH, D], bf16)
    pwon_t = din("pool_wo_n", [D, D], bf16)
    out_u = nc.dram_tensor("out_u", [D, 1], f32, kind="ExternalOutput").ap()
    out_f = nc.dram_tensor("out_f", [128, 4], f32, kind="ExternalOutput").ap()

    W2T = 11  # k-tiles of padded IFF (11x128)
    IFFP = 11 * 128  # 1408, zero-padded from 1365

    with tile.TileContext(nc) as tc:
        with tc.tile_pool(name="cst", bufs=1) as cst, \
             tc.tile_pool(name="wp", bufs=2) as wp, \
             tc.tile_pool(name="ac", bufs=2) as ac, \
             tc.tile_pool(name="ps", bufs=2, space="PSUM") as pp, \
             tc.tile_pool(name="dramp", bufs=2, space="DRAM") as dramp:

            ident = cst.tile([128, 128], bf16, name="ident")
            make_identity(nc, ident[:])
            ones128 = cst.tile([128, 1], bf16, name="ones128")
            nc.vector.memset(ones128[:], 1.0)
            ones1 = cst.tile([1, 128], bf16, name="ones1")
            nc.vector.memset(ones1[:], 1.0)
            epsc = cst.tile([128, 1], f32, name="epsc")
            nc.vector.memset(epsc[:], 1e-5)
            oi512 = cst.tile([128, 1], bf16, name="oi512")
            nc.vector.memset(oi512[:], 1.0 / 512)
            oi1024 = cst.tile([128, 1], bf16, name="oi1024")
            nc.vector.memset(oi1024[:], 1.0 / 1024)
            oi8 = cst.tile([128, 2, 1], f8, name="oi8")
            nc.vector.memset(oi8[:], 1.0 / 512)
            dum = cst.tile([1, 1], f32, name="dum")

            def preload(func):
                # dummy ACT op so the table-load pass hoists the switch
                # into an idle window instead of the LN critical chain
                nc.scalar.activation(out=dum[:], in_=epsc[0:1, :],
                                     func=func)

            # ---------- helpers ----------
            def ln_stats(xs, T, C, tag, want_mB=False, want_col=False,
                         col_chunks=None, qpair=None):
                """Deferred-LN stats over C*128 features, T tokens.

                xs: C bf16 [128,T] APs (raw pre-LN values; any common
                power-of-2 scale -- rstd absorbs it).
                Returns dict: rB [128,T] bf16 rstd broadcast; mursn [1,T]
                bf16 = -mu*rstd (correction-matmul operand); mBn [128,T]
                bf16 broadcast of mursn (want_mB); rC [128,k] bf16 rstd
                column chunks (want_col).
                """
                oi = {4: oi512, 8: oi1024}[C]
                S = pp.tile([1, T], f32, tag="st", bufs=2, name=f"S{tag}")
                Q = pp.tile([1, T], f32, tag="st", bufs=2, name=f"Q{tag}")
                if qpair is not None:
                    xs = [qpair[c // 2][:, c % 2, :] for c in range(C)]
                    for c2 in range(C // 2):
                        nc.tensor.matmul(S[:], oi8[:], qpair[c2][:],
                                         start=(c2 == 0),
                                         stop=(c2 == C // 2 - 1),
                                         perf_mode=DR)
                else:
                    for c in range(C):
                        nc.tensor.matmul(S[:], oi[:], xs[c],
                                         start=(c == 0), stop=(c == C - 1))
                for c in range(C):
                    x2 = ac.tile([128, T], bf16, tag="lnx2", bufs=3,
                                 name="lnx2")
                    nc.vector.tensor_mul(out=x2[:], in0=xs[c], in1=xs[c])
                    nc.tensor.matmul(Q[:], oi[:], x2[:],
                                     start=(c == 0), stop=(c == C - 1))
                m2 = ac.tile([1, T], f32, tag="lnst", bufs=6, name="lnm2")
                nc.scalar.activation(out=m2[:], in_=S[:], func=AF.Square)
                var = ac.tile([1, T], f32, tag="lnst", bufs=6, name="lnvar")
                nc.vector.tensor_sub(out=var[:], in0=Q[:], in1=m2[:])
                lnv = ac.tile([1, T], f32, tag="lnst", bufs=6, name="lnlnv")
                nc.scalar.activation(out=lnv[:], in_=var[:], func=AF.Ln,
                                     bias=epsc[0:1, :])
                pairb = ac.tile([1, 2 * T], bf16, tag="lnpr", bufs=3,
                                name=f"pair{tag}")
                rstd = pairb[:, 0:T]
                nc.scalar.activation(out=rstd, in_=lnv[:], func=AF.Exp,
                                     scale=-0.5)
                nc.vector.scalar_tensor_tensor(
                    out=pairb[:, T:2 * T], in0=S[:], scalar=-1.0,
                    in1=rstd, op0=OP.mult, op1=OP.mult)
                res = {"mursn": pairb[:, T:2 * T], "rstd": rstd}
                BR = pp.tile([128, T], f32, tag="st", bufs=2, name=f"BR{tag}")
                nc.tensor.matmul(BR[:], ones1[:], rstd, start=True, stop=True)
                rB = ac.tile([128, T], bf16, tag="lnrB", bufs=2, name="lnrB")
                nc.vector.tensor_copy(out=rB[:], in_=BR[:])
                res["rB"] = rB
                if want_mB:
                    BM = pp.tile([128, T], f32, tag="st", bufs=2,
                                 name=f"BM{tag}")
                    nc.tensor.matmul(BM[:], ones1[:], pairb[:, T:2 * T],
                                     start=True, stop=True)
                    mBn = ac.tile([128, T], bf16, tag="lnmB", bufs=2,
                                  name="lnmB")
                    nc.vector.tensor_copy(out=mBn[:], in_=BM[:])
                    res["mBn"] = mBn
                if want_col:
                    k = len(col_chunks)
                    pc = pp.tile([128, k], f32, tag="st", bufs=2,
                                 name=f"pc{tag}")
                    for i, (a, b) in enumerate(col_chunks):
                        m = b - a
                        nc.tensor.matmul(pc[0:m, i:i + 1],
                                         pairb[0:1, a:b], ones1[0:1, 0:1],
                                         start=True, stop=True)
                    rC = ac.tile([128, k], f32, tag="lnrC", bufs=2,
                                 name="lnrC")
                    nc.vector.tensor_copy(out=rC[:], in_=pc[:])
                    res["rC"] = rC
                return res

            def load_cols(dram_ap, n, tag, rows=128):
                ts = []
                for c in range(n):
                    t = wp.tile([rows, 1], f32, tag=f"{tag}{c}", bufs=1,
                                name=f"{tag}{c}")
                    nc.sync.dma_start(out=t[:],
                                      in_=dram_ap[rows * c:rows * (c + 1), :])
                    ts.append(t)
                return ts

            def ln_fm(xs, T, gs, bs, out_views):
                """Materialized feature-major layernorm (embed LN2 only)."""
                C = len(xs)
                inv = 1.0 / (128 * C)
                xbs = []
                S = pp.tile([1, T], f32, tag="g", name="lnS")
                Q = pp.tile([1, T], f32, tag="g", name="lnQ")
                for c in range(C):
                    xb = ac.tile([128, T], bf16, tag="lnxb", bufs=10,
                                 name="lnxb")
                    nc.vector.tensor_copy(out=xb[:], in_=xs[c])
                    xbs.append(xb)
                    nc.tensor.matmul(S[:], ones128[:], xb[:],
                                     start=(c == 0), stop=(c == C - 1))
                for c in range(C):
                    x2 = ac.tile([128, T], bf16, tag="lnx2", bufs=3,
                                 name="lnx2")
                    nc.vector.tensor_mul(out=x2[:], in0=xbs[c][:],
                                         in1=xbs[c][:])
                    nc.tensor.matmul(Q[:], ones128[:], x2[:],
                                     start=(c == 0), stop=(c == C - 1))
                mu = ac.tile([1, T], f32, tag="lnst", bufs=6, name="lnmu")
                nc.scalar.mul(out=mu[:], in_=S[:], mul=inv)
                m2 = ac.tile([1, T], f32, tag="lnst", bufs=6, name="lnm2")
                nc.vector.tensor_mul(out=m2[:], in0=mu[:], in1=mu[:])
                var = ac.tile([1, T], f32, tag="lnst", bufs=6, name="lnvar")
                nc.scalar.mul(out=var[:], in_=Q[:], mul=inv)
                nc.vector.tensor_sub(out=var[:], in0=var[:], in1=m2[:])
                rstd = ac.tile([1, T], f32, tag="lnst", bufs=6, name="lnrstd")
                nc.scalar.activation(out=rstd[:], in_=var[:], func=AF.Ln,
                                     bias=epsc[0:1, :])
                nc.scalar.activation(out=rstd[:], in_=rstd[:], func=AF.Exp,
                                     scale=-0.5)
                murs = ac.tile([1, T], f32, tag="lnst", bufs=6, name="lnmurs")
                nc.vector.tensor_mul(out=murs[:], in0=mu[:], in1=rstd[:])
                rb = ac.tile([1, T], bf16, tag="lnsb", bufs=4, name="lnrb")
                nc.vector.tensor_copy(out=rb[:], in_=rstd[:])
                mb = ac.tile([1, T], bf16, tag="lnsb", bufs=4, name="lnmb")
                nc.vector.tensor_copy(out=mb[:], in_=murs[:])
                BR = pp.tile([128, T], f32, tag="g", name="lnBR")
                nc.tensor.matmul(BR[:], ones1[:], rb[:], start=True,
                                 stop=True)
                BM = pp.tile([128, T], f32, tag="g", name="lnBM")
                nc.tensor.matmul(BM[:], ones1[:], mb[:], start=True,
                                 stop=True)
                rB = ac.tile([128, T], bf16, tag="lnrB", bufs=2, name="lnrB")
                nc.vector.tensor_copy(out=rB[:], in_=BR[:])
                mB = ac.tile([128, T], bf16, tag="lnmB", bufs=2, name="lnmB")
                nc.vector.tensor_copy(out=mB[:], in_=BM[:])
                for c in range(C):
                    t1 = ac.tile([128, T], bf16, tag="lnt1", bufs=2,
                                 name="lnt1")
                    nc.vector.tensor_mul(out=t1[:], in0=xbs[c][:], in1=rB[:])
                    nc.vector.tensor_sub(out=t1[:], in0=t1[:], in1=mB[:])
                    nc.vector.tensor_scalar(out=out_views[c], in0=t1[:],
                                            scalar1=gs[c][:],
                                            scalar2=bs[c][:], op0=OP.mult,
                                            op1=OP.add)

            tok_chunks = [(0, 128), (128, 256), (256, 384), (384, 400)]
            rg = [[0, 1, 2, 3], [4, 5, 6, 7]]

            # ---------- embed ----------
            xeT = ac.tile([128, 8, OWN], bf16, tag="xe", bufs=1, name="xeT")
            nc.sync.dma_start(out=xeT[:],
                              in_=x_t.rearrange("(c p) t -> p c t", c=8))
            xe = [xeT[:, c, :] for c in range(8)]
            ewT = wp.tile([128, 8, D], f8, tag="ew", bufs=1, name="ewT")
            nc.sync.dma_start(out=ewT[:],
                              in_=ew_t.rearrange("(c p) f -> p c f", c=8))
            ebs = load_cols(ebias_t, 4, "ebias")
            eg2s = load_cols(eg2, 4, "eg2")
            eb2s = load_cols(eb2, 4, "eb2")

            preload(AF.Exp)
            est = ln_stats(list(xe), OWN, 8, "emb", want_mB=True)
            xeq = [ac.tile([128, 2, OWN], f8, tag=f"xeq{c2}", bufs=1,
                           name=f"xeq{c2}") for c2 in range(4)]
            for c in range(8):
                tmp = ac.tile([128, OWN], bf16, tag="eyt", bufs=2, name="eyt")
                nc.vector.tensor_mul(out=tmp[:], in0=xe[c], in1=est["rB"][:])
                nc.vector.tensor_add(out=xeq[c // 2][:, c % 2, :],
                                     in0=tmp[:], in1=est["mBn"][:])

            t2 = []
            for mc in range(4):
                ps = pp.tile([128, OWN], f32, tag="kv", bufs=4,
                             name=f"embp{mc}")
                for c2 in range(4):
                    nc.tensor.matmul(
                        ps[:], ewT[:, 2 * c2:2 * c2 + 2,
                                   128 * mc:128 * (mc + 1)],
                        xeq[c2][:], start=(c2 == 0), stop=(c2 == 3),
                        perf_mode=DR)
                t = ac.tile([128, OWN], f32, tag="t2", bufs=4, name=f"t2{mc}")
                nc.vector.tensor_scalar_add(out=t[:], in0=ps[:],
                                            scalar1=ebs[mc][:])
                t2.append(t[:])

            tok = [ac.tile([128, TOK], bf16, tag=f"tok{c}", bufs=1,
                           name=f"tok{c}") for c in range(4)]
            ln_fm(t2, OWN, eg2s, eb2s, [tok[c][:, 0:OWN] for c in range(4)])
            for c in range(4):
                nc.sync.dma_start(out=tok[c][:, OWN:TOK],
                                  in_=fus_t[128 * c:128 * (c + 1), :])

            def resid_and_cast(psO_prev):
                """tok += psO/256 and fp8 cast, interleaved per chunk so
                tokq pair 0 lands as early as possible."""
                tq = [ac.tile([128, 2, TOK], f8, tag=f"tq{c2}", bufs=2,
                              name=f"tq{c2}") for c2 in range(2)]
                for c in range(4):
                    if psO_prev is not None:
                        nc.vector.scalar_tensor_tensor(
                            out=tok[c][:], in0=psO_prev[c][:],
                            scalar=1.0 / 256, in1=tok[c][:],
                            op0=OP.mult, op1=OP.add)
                    nc.vector.tensor_copy(out=tq[c // 2][:, c % 2, :],
                                          in_=tok[c][:])
                return tq

            # ---------- layers ----------
            psO_prev = None
            for l in range(DEPTH):
                wqT = wp.tile([128, 4, D], f8, tag="wq", bufs=1, name="wqT")
                nc.sync.dma_start(out=wqT[:],
                                  in_=wq_t[l].rearrange("(c p) f -> p c f",
                                                        c=4))
                wkvT = wp.tile([128, 4, 2 * D], f8, tag="wkv", bufs=2,
                               name="wkvT")
                nc.sync.dma_start(out=wkvT[:],
                                  in_=wkv_t[l].rearrange("(c p) f -> p c f",
                                                         c=4))
                wohT = wp.tile([DH, H, D], bf16, tag="woh", bufs=1,
                               name="wohT")
                nc.sync.dma_start(out=wohT[:],
                                  in_=woh_t[l].rearrange("h d f -> d h f"))
                wonT = wp.tile([128, 4, D], bf16, tag="won", bufs=1,
                               name="wonT")
                nc.sync.dma_start(out=wonT[:],
                                  in_=won_t[l].rearrange("(c p) f -> p c f",
                                                         c=4))
                won = [wonT[:, c, :] for c in range(4)]
                w1T = wp.tile([128, 4, 2 * IFFP], f8, tag="w1", bufs=1,
                              name="w1T")
                nc.sync.dma_start(out=w1T[:],
                                  in_=w1_t[l].rearrange("(c p) f -> p c f",
                                                        c=4))
                w2T_ = wp.tile([128, W2T, D], f8, tag="w2", bufs=1,
                               name="w2T_")
                nc.sync.dma_start(out=w2T_[:],
                                  in_=w2_t[l].rearrange("(j p) f -> p j f",
                                                        j=W2T))
                wbk = wp.tile([1, D], bf16, tag="wbk", bufs=2, name="wbk")
                nc.sync.dma_start(out=wbk[:], in_=wbar_t[l, 0:1, :])
                wbv = wp.tile([1, D], bf16, tag="wbv", bufs=2, name="wbv")
                nc.sync.dma_start(out=wbv[:], in_=wbar_t[l, 1:2, :])
                wbq = wp.tile([1, D], bf16, tag="wbq", bufs=2, name="wbq")
                nc.sync.dma_start(out=wbq[:], in_=wbar_t[l, 2:3, :])

                tokq = resid_and_cast(psO_prev)
                psO_prev = None

                # raw K^T / q^T projections (start immediately off tokq)
                ktp = []
                for mc in range(4):
                    ps = pp.tile([128, OWN], f32, tag="kv", bufs=4,
                                 name=f"kt{mc}")
                    for c2 in range(2):
                        nc.tensor.matmul(
                            ps[:], wkvT[:, 2 * c2:2 * c2 + 2,
                                        128 * mc:128 * (mc + 1)],
                            tokq[c2][:, :, 0:OWN],
                            start=(c2 == 0), stop=False, perf_mode=DR)
                    ktp.append(ps)
                qfp = pp.tile([128, 4, FUS], f32, tag="g", name="qfp")
                for mc in range(4):
                    for c2 in range(2):
                        nc.tensor.matmul(
                            qfp[:, mc, :], wqT[:, 2 * c2:2 * c2 + 2,
                                               128 * mc:128 * (mc + 1)],
                            tokq[c2][:, :, OWN:TOK],
                            start=(c2 == 0), stop=False, perf_mode=DR)

                # LN1 stats run concurrently with the raw projections
                st1 = ln_stats(None, TOK, 4, f"l1{l}", want_col=True,
                               col_chunks=tok_chunks, qpair=tokq)

                # corrections + rstd applies
                kt = []
                for mc in range(4):
                    nc.tensor.matmul(ktp[mc][:],
                                     wbk[:, 128 * mc:128 * (mc + 1)],
                                     st1["mursn"][:, 0:OWN],
                                     start=False, stop=True)
                    s = ac.tile([128, OWN], bf16, tag=f"kt{mc}", bufs=1,
                                name=f"ktb{mc}")
                    nc.vector.tensor_mul(out=s[:], in0=ktp[mc][:],
                                         in1=st1["rB"][:, 0:OWN])
                    kt.append(s)
                qf = []
                for mc in range(4):
                    nc.tensor.matmul(qfp[:, mc, :],
                                     wbq[:, 128 * mc:128 * (mc + 1)],
                                     st1["mursn"][:, OWN:TOK],
                                     start=False, stop=True)
                    s = ac.tile([128, 32], bf16, tag=f"qf{mc}", bufs=1,
                                name=f"qfb{mc}")
                    nc.vector.memset(s[:, FUS:32], 0.0)
                    nc.vector.tensor_mul(out=s[:, 0:FUS], in0=qfp[:, mc, :],
                                         in1=st1["rB"][:, OWN:TOK])
                    qf.append(s)

                # V token-major raw + correction, rstd col-scale on copy
                V = []
                for i, (a, b) in enumerate(tok_chunks):
                    m = b - a
                    ps = pp.tile([128, D], f32, tag="kv", bufs=4,
                                 name=f"v{i}")
                    for c2 in range(2):
                        nc.tensor.matmul(ps[0:m, :],
                                         tokq[c2][:, :, a:b],
                                         wkvT[:, 2 * c2:2 * c2 + 2, D:2 * D],
                                         start=(c2 == 0), stop=False,
                                         perf_mode=DR)
                    nc.tensor.matmul(ps[0:m, :], st1["mursn"][:, a:b],
                                     wbv[:], start=False, stop=True)
                    s = ac.tile([128, D], bf16, tag=f"V{i}", bufs=1,
                                name=f"Vb{i}")
                    nc.scalar.activation(out=s[0:m, :], in_=ps[0:m, :],
                                         func=AF.Copy,
                                         scale=st1["rC"][0:m, i:i + 1])
                    V.append(s)

                # V column sums: own (exchanged early) + fusion (local)
                vsf = pp.tile([128, 8], f32, tag="g", name="vsf")
                for c in range(4):
                    for j in range(3):
                        nc.tensor.matmul(vsf[:, c:c + 1],
                                         V[j][:, 128 * c:128 * (c + 1)],
                                         ones128[:], start=(j == 0),
                                         stop=(j == 2))
                for c in range(4):
                    nc.tensor.matmul(vsf[:, 4 + c:5 + c],
                                     V[3][0:FUS, 128 * c:128 * (c + 1)],
                                     ones128[0:FUS, :], start=True, stop=True)
                Pv = ac.tile([128, 4], f32, tag="Pv", bufs=2, name="Pv")
                nc.vector.tensor_copy(out=Pv[:], in_=vsf[:, 0:4])
                vfu = ac.tile([128, 4], f32, tag="vfu", bufs=2, name="vfu")
                nc.vector.tensor_copy(out=vfu[:], in_=vsf[:, 4:8])
                pin1 = dramp.tile([128, 4], f32, tag="pin1", bufs=2,
                                  name="pin1")
                nc.sync.dma_start(out=pin1[:], in_=Pv[:])
                Rv = ac.tile([128, 4, 4], f32, tag="Rv", bufs=2, name="Rv")
                if use_cc:
                    pout1 = dramp.tile([4 * 128, 4], f32, tag="pout1",
                                       bufs=2, name="pout1")
                    nc.gpsimd.collective_compute(
                        "AllGather", OP.bypass, replica_groups=rg,
                        ins=[pin1.opt()], outs=[pout1.opt()])
                    nc.sync.dma_start(
                        out=Rv[:],
                        in_=pout1.rearrange("(r p) f -> p r f", r=4))
                else:
                    nc.sync.dma_start(
                        out=Rv[:],
                        in_=pin1.rearrange("(r p) f -> p r f", r=1)
                        .to_broadcast((128, 4, 4)))

                # scores + exp (+row sums); kt/qf carry 2^16 scale
                E, lacc = [], []
                for t in range(2):
                    sp = pp.tile([128, OWN], f32, tag="g", name=f"sp{t}")
                    for i in range(4):
                        h = 4 * t + i
                        ch, base = h // 2, (h % 2) * 64
                        nc.tensor.matmul(sp[32 * i:32 * i + 32, :],
                                         qf[ch][base:base + 64, 0:32],
                                         kt[ch][base:base + 64, :],
                                         start=True, stop=True,
                                         tile_position=(base, 32 * i))
                    e = ac.tile([128, OWN], bf16, tag=f"e{t}", bufs=1,
                                name=f"e{t}")
                    la = ac.tile([128, 1], f32, tag=f"la{t}", bufs=2,
                                 name=f"la{t}")
                    nc.scalar.activation(out=e[:], in_=sp[:], func=AF.Exp,
                                         scale=1.0 / 65536, accum_out=la[:])
                    E.append(e)
                    lacc.append(la)

                ET = [[None] * 3 for _ in range(2)]
                for t in range(2):
                    for j in range(3):
                        pt = pp.tile([128, 128], bf16, tag="g",
                                     name=f"et{t}{j}")
                        nc.tensor.transpose(pt[:],
                                            E[t][:, 128 * j:128 * (j + 1)],
                                            ident[:])
                        s = ac.tile([128, 128], bf16, tag=f"ET{t}{j}",
                                    bufs=1, name=f"ETb{t}{j}")
                        nc.vector.tensor_copy(out=s[:], in_=pt[:])
                        ET[t][j] = s

                # payload P = [l0, l1, ACC0, ACC1]
                P = ac.tile([128, 130], f32, tag="P", bufs=2, name="P")
                nc.vector.tensor_copy(out=P[:, 0:1], in_=lacc[0][:])
                nc.vector.tensor_copy(out=P[:, 1:2], in_=lacc[1][:])
                accp = pp.tile([128, 2, 64], f32, tag="g", name="accp")
                for t in range(2):
                    for i in range(4):
                        h = 4 * t + i
                        for j in range(3):
                            nc.tensor.matmul(accp[32 * i:32 * i + 32, t, :],
                                             ET[t][j][:, 32 * i:32 * i + 32],
                                             V[j][:, DH * h:DH * (h + 1)],
                                             start=(j == 0), stop=(j == 2),
                                             tile_position=(0, 32 * i))
                    nc.vector.tensor_copy(out=P[:, 2 + 64 * t:66 + 64 * t],
                                          in_=accp[:, t, :])
                pin2 = dramp.tile([128, 130], f32, tag="pin2", bufs=2,
                                  name="pin2")
                nc.sync.dma_start(out=pin2[:], in_=P[:])
                R2 = ac.tile([128, 4, 130], f32, tag="R2", bufs=2, name="R2")
                if use_cc:
                    pout2 = dramp.tile([4 * 128, 130], f32, tag="pout2",
                                       bufs=2, name="pout2")
                    nc.gpsimd.collective_compute(
                        "AllGather", OP.bypass, replica_groups=rg,
                        ins=[pin2.opt()], outs=[pout2.opt()])
                    nc.sync.dma_start(
                        out=R2[:],
                        in_=pout2.rearrange("(r p) f -> p r f", r=4))
                else:
                    nc.sync.dma_start(
                        out=R2[:],
                        in_=pin2.rearrange("(r p) f -> p r f", r=1)
                        .to_broadcast((128, 4, 130)))

                # uniform delta from the early vsum exchange
                vT2 = ac.tile([128, 2, 4], f32, tag="cmb", bufs=3,
                              name="vT2")
                nc.vector.tensor_add(out=vT2[:], in0=Rv[:, 0:2, :],
                                     in1=Rv[:, 2:4, :])
                vsb = ac.tile([128, 4], bf16, tag="vsb", bufs=2, name="vsb")
                nc.vector.scalar_tensor_tensor(
                    out=vsb[:], in0=vT2[:, 0, :], scalar=1.0,
                    in1=vT2[:, 1, :], op0=OP.bypass, op1=OP.add)
                nc.vector.tensor_add(out=vsb[:], in0=vsb[:], in1=vfu[:])
                dup = pp.tile([128, 4], f32, tag="g", name="dup")
                for c in range(4):
                    for kc in range(4):
                        nc.tensor.matmul(dup[:, c:c + 1],
                                         won[kc][:, 128 * c:128 * (c + 1)],
                                         vsb[:, kc:kc + 1],
                                         start=(kc == 0), stop=(kc == 3))
                dub = ac.tile([128, 4], f32, tag="dub", bufs=2, name="dub")
                nc.vector.tensor_copy(out=dub[:], in_=dup[:])
                dus = [dub[:, c:c + 1] for c in range(4)]

                # fusion delta, directly feature-major [128, 4c, 16]
                T01 = ac.tile([128, 130], f32, tag="cmb2", bufs=3,
                              name="T01")
                nc.vector.tensor_add(out=T01[:], in0=R2[:, 0, :],
                                     in1=R2[:, 1, :])
                T23 = ac.tile([128, 130], f32, tag="cmb2", bufs=3,
                              name="T23")
                nc.vector.tensor_add(out=T23[:], in0=R2[:, 2, :],
                                     in1=R2[:, 3, :])
                PT = ac.tile([128, 130], f32, tag="cmb2", bufs=3, name="PT")
                nc.vector.tensor_add(out=PT[:], in0=T01[:], in1=T23[:])
                linv = ac.tile([128, 2], f32, tag="linv", bufs=2,
                               name="linv")
                nc.vector.reciprocal(out=linv[:], in_=PT[:, 0:2])
                ofT = []
                for t in range(2):
                    of = ac.tile([128, 64], bf16, tag=f"of{t}", bufs=1,
                                 name=f"of{t}")
                    nc.vector.tensor_scalar_mul(
                        out=of[:], in0=PT[:, 2 + 64 * t:66 + 64 * t],
                        scalar1=linv[:, t:t + 1])
                    pt = pp.tile([64, 128], bf16, tag="g", name=f"oft{t}")
                    nc.tensor.transpose(pt[:], of[:], ident[:])
                    s = ac.tile([64, 128], bf16, tag=f"ofT{t}", bufs=1,
                                name=f"ofTb{t}")
                    nc.vector.tensor_copy(out=s[:], in_=pt[:])
                    ofT.append(s)
                dfT = pp.tile([128, 4, FUS], f32, tag="g", name="dfT")
                for h in range(H):
                    t, i = h // 4, h % 4
                    for c in range(4):
                        nc.tensor.matmul(
                            dfT[:, c, :],
                            wohT[:, h, 128 * c:128 * (c + 1)],
                            ofT[t][:, 32 * i:32 * i + FUS],
                            start=(h == 0), stop=(h == H - 1))

                # LN2: residual applied in place, then stats + fp8 y
                for c in range(4):
                    nc.vector.tensor_scalar_add(out=tok[c][:, 0:OWN],
                                                in0=tok[c][:, 0:OWN],
                                                scalar1=dus[c])
                    nc.vector.tensor_add(out=tok[c][:, OWN:TOK],
                                         in0=tok[c][:, OWN:TOK],
                                         in1=dfT[:, c, :])
                st2 = ln_stats([tok[c][:] for c in range(4)], TOK, 4,
                               f"l2{l}", want_mB=True)
                preload(AF.Gelu)
                xn2q = [ac.tile([128, 2, TOK], f8, tag=f"x2q{c2}", bufs=1,
                                name=f"x2q{c2}") for c2 in range(2)]
                for c in range(4):
                    tmp = ac.tile([128, TOK], bf16, tag="yt2", bufs=2,
                                  name="yt2")
                    nc.vector.tensor_mul(out=tmp[:], in0=tok[c][:],
                                         in1=st2["rB"][:])
                    nc.vector.tensor_add(out=xn2q[c // 2][:, c % 2, :],
                                         in0=tmp[:], in1=st2["mBn"][:])

                # GEGLU FF in fp8 DoubleRow
                gtq = [ac.tile([128, 2, TOK], f8, tag=f"gtq{jj}", bufs=1,
                               name=f"gtq{jj}") for jj in range(5)]
                gt10 = ac.tile([128, TOK], f8, tag="gt10", bufs=1,
                               name="gt10")
                for j in range(W2T):
                    a = 128 * j
                    px = pp.tile([128, TOK], f32, tag="kv", bufs=4,
                                 name=f"fx{j}")
                    pg = pp.tile([128, TOK], f32, tag="st", bufs=2,
                                 name=f"fg{j}")
                    for c2 in range(2):
                        nc.tensor.matmul(
                            px[:], w1T[:, 2 * c2:2 * c2 + 2, a:a + 128],
                            xn2q[c2][:], start=(c2 == 0), stop=(c2 == 1),
                            perf_mode=DR)
                    for c2 in range(2):
                        nc.tensor.matmul(
                            pg[:], w1T[:, 2 * c2:2 * c2 + 2,
                                       IFFP + a:IFFP + a + 128],
                            xn2q[c2][:], start=(c2 == 0), stop=(c2 == 1),
                            perf_mode=DR)
                    gg = ac.tile([128, TOK], bf16, tag="gg", bufs=3,
                                 name=f"gg{j}")
                    nc.scalar.activation(out=gg[:], in_=pg[:], func=AF.Gelu,
                                         scale=1.0 / 128)
                    gdst = gtq[j // 2][:, j % 2, :] if j < 10 else gt10[:]
                    nc.vector.tensor_mul(out=gdst, in0=gg[:], in1=px[:])
                    if j == W2T - 1:
                        preload(AF.Exp)
                psO_prev = []
                for c in range(4):
                    psO = pp.tile([128, TOK], f32, tag="kv", bufs=4,
                                  name=f"fo{c}")
                    for jj in range(5):
                        nc.tensor.matmul(
                            psO[:], w2T_[:, 2 * jj:2 * jj + 2,
                                         128 * c:128 * (c + 1)],
                            gtq[jj][:], start=(jj == 0), stop=False,
                            perf_mode=DR)
                    nc.tensor.matmul(psO[:],
                                     w2T_[:, 10, 128 * c:128 * (c + 1)],
                                     gt10[:], start=False, stop=True)
                    psO_prev.append(psO)

            # ---------- pool ----------
            pwkvT = wp.tile([128, 4, 2 * D], f8, tag="pwkv", bufs=1,
                            name="pwkvT")
            nc.sync.dma_start(out=pwkvT[:],
                              in_=pwkv_t.rearrange("(c p) f -> p c f", c=4))
            pbk = wp.tile([1, D], bf16, tag="pbk", bufs=1, name="pbk")
            nc.sync.dma_start(out=pbk[:], in_=pbar_t[0:1, :])
            pbv = wp.tile([1, D], bf16, tag="pbv", bufs=1, name="pbv")
            nc.sync.dma_start(out=pbv[:], in_=pbar_t[1:2, :])
            pwoh = []
            for h in range(H):
                t = wp.tile([DH, D], bf16, tag=f"woh{h}", bufs=1,
                            name=f"pwoh{h}")
                nc.sync.dma_start(out=t[:], in_=pwoh_t[h])
                pwoh.append(t)
            pwon = []
            for c in range(4):
                t = wp.tile([128, D], bf16, tag=f"pwon{c}", bufs=1,
                            name=f"pwon{c}")
                nc.sync.dma_start(out=t[:],
                                  in_=pwon_t[128 * c:128 * (c + 1), :])
                pwon.append(t)
            pq2s = load_cols(pq2_t, 4, "pq2")

            tokq = resid_and_cast(psO_prev)
            stp = ln_stats(None, TOK, 4, "pool",
                           want_col=True, col_chunks=tok_chunks, qpair=tokq)

            # V_pool token-major
            Vp = []
            for i, (a, b) in enumerate(tok_chunks):
                m = b - a
                ps = pp.tile([128, D], f32, tag="kv", bufs=4, name=f"pv{i}")
                for c2 in range(2):
                    nc.tensor.matmul(ps[0:m, :], tokq[c2][:, :, a:b],
                                     pwkvT[:, 2 * c2:2 * c2 + 2, D:2 * D],
                                     start=(c2 == 0), stop=False,
                                     perf_mode=DR)
                nc.tensor.matmul(ps[0:m, :], stp["mursn"][:, a:b], pbv[:],
                                 start=False, stop=True)
                s = ac.tile([128, D], bf16, tag=f"V{i}", bufs=1,
                            name=f"pVb{i}")
                nc.scalar.activation(out=s[0:m, :], in_=ps[0:m, :],
                                     func=AF.Copy,
                                     scale=stp["rC"][0:m, i:i + 1])
                Vp.append(s)

            # pool vsum exchange (Vp carries 256x scale; pwon descales)
            pvsf = pp.tile([128, 8], f32, tag="g", name="pvsf")
            for c in range(4):
                for j in range(3):
                    nc.tensor.matmul(pvsf[:, c:c + 1],
                                     Vp[j][:, 128 * c:128 * (c + 1)],
                                     ones128[:], start=(j == 0),
                                     stop=(j == 2))
            for c in range(4):
                nc.tensor.matmul(pvsf[:, 4 + c:5 + c],
                                 Vp[3][0:FUS, 128 * c:128 * (c + 1)],
                                 ones128[0:FUS, :], start=True, stop=True)
            Pp = ac.tile([128, 4], f32, tag="Pp", bufs=2, name="Pp")
            nc.vector.tensor_copy(out=Pp[:], in_=pvsf[:, 0:4])
            pvfu = ac.tile([128, 4], f32, tag="vfu", bufs=2, name="pvfu")
            nc.vector.tensor_copy(out=pvfu[:], in_=pvsf[:, 4:8])
            pinp = dramp.tile([128, 4], f32, tag="pinp", bufs=1, name="pinp")
            nc.sync.dma_start(out=pinp[:], in_=Pp[:])
            Rpa = ac.tile([128, 4, 4], f32, tag="Rv", bufs=2, name="Rpa")
            if use_cc:
                poutp = dramp.tile([4 * 128, 4], f32, tag="poutp", bufs=1,
                                   name="poutp")
                nc.gpsimd.collective_compute(
                    "AllGather", OP.bypass, replica_groups=rg,
                    ins=[pinp.opt()], outs=[poutp.opt()])
                nc.sync.dma_start(
                    out=Rpa[:],
                    in_=poutp.rearrange("(r p) f -> p r f", r=4))
            else:
                nc.sync.dma_start(
                    out=Rpa[:],
                    in_=pinp.rearrange("(r p) f -> p r f", r=1)
                    .to_broadcast((128, 4, 4)))
            pT2 = ac.tile([128, 2, 4], f32, tag="cmb", bufs=3, name="pT2")
            nc.vector.tensor_add(out=pT2[:], in0=Rpa[:, 0:2, :],
                                 in1=Rpa[:, 2:4, :])
            pvsb = ac.tile([128, 4], bf16, tag="vsb", bufs=2, name="pvsb")
            nc.vector.scalar_tensor_tensor(
                out=pvsb[:], in0=pT2[:, 0, :], scalar=1.0,
                in1=pT2[:, 1, :], op0=OP.bypass, op1=OP.add)
            nc.vector.tensor_add(out=pvsb[:], in0=pvsb[:], in1=pvfu[:])
            pdup = pp.tile([128, 4], f32, tag="g", name="pdup")
            for c in range(4):
                for kc in range(4):
                    nc.tensor.matmul(pdup[:, c:c + 1],
                                     pwon[kc][:, 128 * c:128 * (c + 1)],
                                     pvsb[:, kc:kc + 1],
                                     start=(kc == 0), stop=(kc == 3))
            pdub = ac.tile([128, 4], f32, tag="du", bufs=2, name="pdub")
            nc.vector.tensor_copy(out=pdub[:], in_=pdup[:])
            for c in range(4):
                nc.sync.dma_start(out=out_u[128 * c:128 * (c + 1), :],
                                  in_=pdub[:, c:c + 1])

            # fusion-key attention for return token 2 (all local)
            kf = []
            kfp = pp.tile([128, 6, FUS], f32, tag="g", name="kfp")
            for mc in range(4):
                for c2 in range(2):
                    nc.tensor.matmul(kfp[:, mc, :],
                                     pwkvT[:, 2 * c2:2 * c2 + 2,
                                           128 * mc:128 * (mc + 1)],
                                     tokq[c2][:, :, OWN:TOK],
                                     start=(c2 == 0), stop=False,
                                     perf_mode=DR)
                nc.tensor.matmul(kfp[:, mc, :],
                                 pbk[:, 128 * mc:128 * (mc + 1)],
                                 stp["mursn"][:, OWN:TOK],
                                 start=False, stop=True)
                s = ac.tile([128, FUS], bf16, tag=f"kf{mc}", bufs=1,
                            name=f"kfb{mc}")
                nc.vector.tensor_mul(out=s[:], in0=kfp[:, mc, :],
                                     in1=stp["rB"][:, OWN:TOK])
                kf.append(s)
            q2 = []
            for mc in range(4):
                s = ac.tile([128, 32], bf16, tag=f"q2{mc}", bufs=1,
                            name=f"q2b{mc}")
                nc.vector.memset(s[:, 1:32], 0.0)
                nc.vector.tensor_copy(out=s[:, 0:1], in_=pq2s[mc][:])
                q2.append(s)
            e2, l2 = [], []
            for t in range(2):
                sp = kfp[:, 4 + t, :]
                for i in range(4):
                    h = 4 * t + i
                    ch, base = h // 2, (h % 2) * 64
                    nc.tensor.matmul(sp[32 * i:32 * i + 32, :],
                                     q2[ch][base:base + 64, 0:32],
                                     kf[ch][base:base + 64, :],
                                     start=True, stop=True,
                                     tile_position=(base, 32 * i))
                e = ac.tile([128, FUS], bf16, tag=f"e2{t}", bufs=1,
                            name=f"e2{t}")
                la = ac.tile([128, 1], f32, tag=f"la{t}", bufs=2,
                             name=f"pla{t}")
                nc.scalar.activation(out=e[:], in_=sp[:], func=AF.Exp,
                                     scale=1.0 / 256, accum_out=la[:])
                e2.append(e)
                l2.append(la)
            e2T = []
            for t in range(2):
                pt = pp.tile([FUS, 128], bf16, tag="g", name=f"pet{t}")
                nc.tensor.transpose(pt[:], e2[t][:], ident[:])
                s = ac.tile([FUS, 128], bf16, tag=f"e2T{t}", bufs=1,
                            name=f"e2Tb{t}")
                nc.vector.tensor_copy(out=s[:], in_=pt[:])
                e2T.append(s)
            ofT2 = []
            pacc2 = pp.tile([128, 2, 64], f32, tag="g", name="pacc2")
            for t in range(2):
                for i in range(4):
                    h = 4 * t + i
                    nc.tensor.matmul(pacc2[32 * i:32 * i + 32, t, :],
                                     e2T[t][:, 32 * i:32 * i + 32],
                                     Vp[3][0:FUS, DH * h:DH * (h + 1)],
                                     start=True, stop=True,
                                     tile_position=(0, 32 * i))
                li = ac.tile([128, 1], f32, tag="linv", bufs=2,
                             name=f"pli{t}")
                nc.vector.reciprocal(out=li[:], in_=l2[t][:])
                of = ac.tile([128, 64], bf16, tag=f"of{t}", bufs=1,
                             name=f"pof{t}")
                nc.vector.tensor_scalar_mul(out=of[:], in0=pacc2[:, t, :],
                                            scalar1=li[:])
                pt = pp.tile([64, 128], bf16, tag="g", name=f"poft{t}")
                nc.tensor.transpose(pt[:], of[:], ident[:])
                s = ac.tile([64, 128], bf16, tag=f"ofT{t}", bufs=1,
                            name=f"pofTb{t}")
                nc.vector.tensor_copy(out=s[:], in_=pt[:])
                ofT2.append(s)
            # P2^T feature-major [128, 4]: 32 matmuls moving 1
            P2 = pp.tile([128, 4], f32, tag="g", name="P2")
            for h in range(H):
                t, i = h // 4, h % 4
                for c in range(4):
                    nc.tensor.matmul(P2[:, c:c + 1],
                                     pwoh[h][:, 128 * c:128 * (c + 1)],
                                     ofT2[t][:, 32 * i:32 * i + 1],
                                     start=(h == 0), stop=(h == H - 1))
            p2s = ac.tile([128, 4], f32, tag="p2s", bufs=1, name="p2s")
            nc.vector.tensor_copy(out=p2s[:], in_=P2[:])
            nc.sync.dma_start(out=out_f[:], in_=p2s[:])

    nc.compile()
    _built[key] = nc
    return nc


def _pad_w1(w1f):
    """[DEPTH, D, 2*IFF] -> [DEPTH, D, 2*1408] with x1/gate blocks padded."""
    out = np.zeros((DEPTH, D, 2 * 1408), np.float64)
    out[:, :, 0:IFF] = w1f[:, :, 0:IFF]
    out[:, :, 1408:1408 + IFF] = w1f[:, :, IFF:2 * IFF]
    return out


def _q8(x, s):
    """fp8e4m3 quantize with a power-of-2 scale folded in."""
    return np.clip(np.asarray(x, np.float64) * s, -224, 224).astype(F8)


def _prep_inputs(inputs):
    """Host-side prep: slice/transpose/cast/quantize per-core input dicts."""
    I = {k: np.asarray(v) for k, v in inputs.items()}
    f32 = np.float32

    def bf(x):
        return np.ascontiguousarray(x).astype(BF)

    def col(x):
        return np.ascontiguousarray(np.asarray(x, f32).reshape(-1, 1))

    scale = DH ** -0.5
    wqf = I["layers_wq"].astype(np.float64) * scale \
        * I["layers_attn_g"].astype(np.float64)[:, :, None]
    wkvf = I["layers_wkv"].astype(np.float64) \
        * I["layers_attn_g"].astype(np.float64)[:, :, None]
    w1f = _pad_w1(I["layers_ff_w1"].astype(np.float64)
                  * I["layers_ff_g"].astype(np.float64)[:, :, None])
    w1f[:, :, 0:1408] *= 8.0       # x1 half
    w1f[:, :, 1408:] *= 128.0      # gate half
    w2f = np.pad(I["layers_ff_w2"].astype(np.float64),
                 ((0, 0), (0, 1408 - IFF), (0, 0))) * 32.0
    pkvf = I["pool_wkv"].astype(np.float64) \
        * I["final_g"].astype(np.float64)[:, None]

    wq_q = _q8(wqf, 256.0)
    wkv_q = _q8(wkvf, 256.0)
    w1_q = np.clip(w1f, -224, 224).astype(F8)
    w2_q = np.clip(w2f, -224, 224).astype(F8)
    pkv_q = _q8(pkvf, 256.0)

    wkv_d = wkv_q.astype(np.float64)
    wq_d = wq_q.astype(np.float64)
    pkv_d = pkv_q.astype(np.float64)
    wbar = np.stack([wkv_d[:, :, 0:D].sum(axis=1),       # k-half
                     wkv_d[:, :, D:2 * D].sum(axis=1),   # v-half
                     wq_d.sum(axis=1)], axis=1)          # q
    pbar = np.stack([pkv_d[:, 0:D].sum(axis=0),
                     pkv_d[:, D:2 * D].sum(axis=0)], axis=0)

    shared = {
        "fus_t": bf(I["fusion_tokens"].astype(np.float64).T),
        "wq": wq_q,
        "wkv": wkv_q,
        "wo_h": bf(I["layers_wo"].reshape(DEPTH, H, DH, D) / 256.0),
        "wo_n": bf(I["layers_wo"] * (1.0 / (NALL * 256.0))),
        "w1": w1_q,
        "w2": w2_q,
        "wbar": bf(wbar),
        "pool_wkv": pkv_q,
        "pool_wbar": bf(pbar),
        "pool_wo_h": bf(I["pool_wo"].reshape(H, DH, D) / 256.0),
        "pool_wo_n": bf(I["pool_wo"] * (1.0 / (NALL * 256.0))),
    }
    # host-side pool query for return token 2 (row 2 = FUSION)
    ret = I["return_tokens"].astype(f32)
    g = I["pool_g"].astype(f32)
    mu = ret.mean(-1, keepdims=True)
    var = ((ret - mu) ** 2).mean(-1, keepdims=True)
    retn = (ret - mu) / np.sqrt(var + 1e-5) * g
    q2 = (retn[2] @ I["pool_wq"].astype(f32)) * scale
    shared["pool_q2"] = col(q2)

    in_maps = []
    for c in range(N_CORES):
        b, q = c // 4, c % 4
        mod = "rna" if q < 2 else "atac"
        x = I[mod][b, (q % 2) * OWN:(q % 2 + 1) * OWN, :]  # [384, 1024]
        m = dict(shared)
        m["x_t"] = bf(x.astype(np.float64).T)
        ewf = I[f"{mod}_w"].astype(np.float64) \
            * I[f"{mod}_ln1_g"].astype(np.float64)[:, None]
        m["emb_w"] = _q8(ewf, 256.0)
        m["emb_b"] = col((I[f"{mod}_b"].astype(np.float64)
                          + I[f"{mod}_ln1_b"].astype(np.float64)
                          @ I[f"{mod}_w"].astype(np.float64)) * 256.0)
        m["eln2_g"] = col(I[f"{mod}_ln2_g"])
        m["eln2_b"] = col(I[f"{mod}_ln2_b"])
        in_maps.append(m)
    return in_maps, ret


def kernel(**inputs):
    from concourse import bass_utils
    nc = build(num_devices=N_CORES, use_cc=True)
    in_maps, ret = _prep_inputs(inputs)
    res = bass_utils.run_bass_kernel_spmd(nc, in_maps,
                                          core_ids=list(range(N_CORES)))
    out = np.zeros((B, 3, D), np.float32)
    for b in range(2):
        r = res.results[4 * b]
        u = r["out_u"][:, 0]
        f = r["out_f"].T.ravel()
        out[b, 0] = u + ret[0]
        out[b, 1] = u + ret[1]
        out[b, 2] = f + ret[2]
    return out
